# revision 5
# baseline (speedup 1.0000x reference)
"""Trainium2 Bass kernel for a dense transformer block (pre-LN, masked attention).

Sharding: data-parallel over batch B=8 across the 8 NeuronCores — each core
processes one full batch element [T=1024, C=1024]; weights are replicated.
No collectives needed.

Per-core dataflow (single NeuronCore), fp8-heavy:
  - LN1 token-major stats; normalized xn scaled x16 (bf16), PE-transposed,
    evicted to fp8 xnT8 [C, T] (e4m3, DVE copy).
  - QKV in fp8 DoubleRow (contraction pairs of 128-blocks): weights
    pre-quantized x512 on host.  Q/K evicted bf16 in true units (ACT, bias
    fused).  V evicted fp8 x32 key-major with a ones(=32) column per head so
    AV's psum row 64 gives the softmax sums.
  - QK^T bf16 (contraction = 64 head dims): the two heads of a pair occupy
    partitions 0-63 / 64-127, issued interleaved so their matmuls run
    concurrently in different row-groups of the PE array (tile_position
    auto-derived).  exp on ScalarE with key-padding mask + ln(8) as bias,
    evicted straight to fp8 st8 (x8).
  - AV in fp8 DoubleRow over key-tile pairs; normalization fused into the
    yT8 eviction (x512, DVE scalar_tensor_tensor with broadcast 1/rowsum).
  - proj in fp8 DoubleRow, eviction fused with residual add (DVE).
  - LN2 -> xn2T8 (same path as LN1).
  - FC1/FC2 in fp8 DoubleRow with same-scale hi/lo weight split (w = fp8(w)
    + fp8(w - fp8(w)): residual lands in e4m3 denormals, so weights carry
    ~bf16 accuracy; both halves accumulate in one PSUM group).  FC1 is
    weight-stationary (each stationary reused for both token halves); FC2
    token-major (each h2T8 stationary reused for both output-column halves).
"""

import os
import sys
import numpy as np
import ml_dtypes

for _p in ("/opt/trn_rl_repo", "/opt/pypackages"):
    if os.path.isdir(_p) and _p not in sys.path:
        sys.path.append(_p)

import concourse.bass as bass
import concourse.mybir as mybir
import concourse.tile as tile
from concourse import bacc
from concourse.bass_utils import run_bass_kernel_spmd
from concourse.masks import make_identity

P = 128
B, T, C = 8, 1024, 1024
NH, HD = 16, 64
FF = 4 * C
EPS = 1e-5
NT = T // P      # 8 token tiles
NCD = C // P     # 8 feature tiles
NFF = FF // P    # 32 ff tiles
N_CORES = 8
MASK_VAL = -30000.0

F32 = mybir.dt.float32
BF16 = mybir.dt.bfloat16
FP8 = mybir.dt.float8e4
AF = mybir.ActivationFunctionType
OP = mybir.AluOpType
DR = mybir.MatmulPerfMode.DoubleRow

bf16 = ml_dtypes.bfloat16
E4 = ml_dtypes.float8_e4m3

# power-of-2 quantization scales
SX = 16.0     # normalized activations (xn, xn2)
SW = 512.0    # weights
SV = 32.0     # v
SS = 8.0      # st = exp(logits)
SY = 512.0    # y (attention out)
S_QKV_EV = 1.0 / (SX * SW)          # q/k eviction: psum -> true units
S_V_EV = SV / (SX * SW)             # v eviction
S_PROJ_EV = 1.0 / (SY * SW)         # proj eviction
S_FC1_EV = SV / (SX * SW)           # fc1 eviction: h2*32 = relu(ps/256 + 32*b)
SH = 32.0     # h2 scale (= SV reuse)
S_FC2_EV = 1.0 / (SH * SW)          # fc2 eviction


def _q8(a):
    return np.clip(a, -240, 240).astype(E4)


def _q8_hilo(a):
    """same-scale hi/lo split: returns (hi, lo) fp8 arrays; hi+lo ~ a."""
    hi = _q8(a)
    lo = _q8(a - hi.astype(np.float32))
    return hi, lo


# --------------------------------------------------------------------------
# host-side preparation: fold LN gains/biases into weights, quantize to fp8
# --------------------------------------------------------------------------
def _host_prep(x, seq_ls, ln1_g, ln1_b, w_qkv, b_qkv, w_proj, b_proj,
               ln2_g, ln2_b, w_fc, b_fc, w_fc2, b_fc2):
    f32 = np.float32
    ln1_g, ln1_b = ln1_g.astype(f32), ln1_b.astype(f32)
    w_qkv = w_qkv.astype(f32)

    wqkv_eff = ln1_g[:, None] * w_qkv                     # [C, 3C]
    bqkv_eff = ln1_b @ w_qkv + b_qkv.astype(f32)          # [3C]
    scale = np.float32(1.0 / np.sqrt(HD))
    wq = wqkv_eff[:, :C] * scale
    bq = bqkv_eff[:C] * scale
    wk = wqkv_eff[:, C:2 * C]
    bk = bqkv_eff[C:2 * C]
    wv = wqkv_eff[:, 2 * C:]
    bv = bqkv_eff[2 * C:]

    bproj_eff = bv @ w_proj.astype(f32) + b_proj.astype(f32)   # [C]

    wfc_eff = ln2_g.astype(f32)[:, None] * w_fc.astype(f32)    # [C, FF]
    bfc_eff = ln2_b.astype(f32) @ w_fc.astype(f32) + b_fc.astype(f32)

    wqk = np.concatenate([wq, wk], axis=1)                # [C, 2C]
    bqk_t = np.concatenate([bq, bk]).reshape(16, P).T.copy()   # [P, 16]
    bfc_t = (bfc_eff * SH).reshape(NFF, P).T.copy()       # [P, 32] (x32)

    # --- fp8 weight layouts ---
    # wqk8 [16, P, 4*2*128]: per head-pair column block mm: (kop, pair, col)
    wqk_s = _q8(wqk * SW)                                 # [C, 2C]
    wqk8 = np.ascontiguousarray(
        wqk_s.reshape(4, 2, P, 16, P)                     # (i, j, p, mm, m)
        .transpose(3, 2, 0, 1, 4)                         # (mm, p, i, j, m)
    ).reshape(16, P, 4 * 2 * P)

    # wv8 [2, P, 4*2*512]: slab n covers output cols n*512..: (kop, pair, col)
    wv_s = _q8(wv * SW)                                   # [C, C]
    wv8 = np.ascontiguousarray(
        wv_s.reshape(4, 2, P, 2, 512)                     # (i, j, p, n, e)
        .transpose(3, 2, 0, 1, 4)                         # (n, p, i, j, e)
    ).reshape(2, P, 4 * 2 * 512)

    # wproj8 [2, P, 4*2*512]
    wp_s = _q8(w_proj.astype(f32) * SW)
    wproj8 = np.ascontiguousarray(
        wp_s.reshape(4, 2, P, 2, 512).transpose(3, 2, 0, 1, 4)
    ).reshape(2, P, 4 * 2 * 512)

    # wfc1 [32, P, 4*2*2*128]: per kk: (kop i, hl, pair j, col m)
    whi, wlo = _q8_hilo(wfc_eff * SW)                     # [C, FF] each
    wfc1 = np.stack([whi, wlo], axis=0)                   # (hl, C, FF)
    wfc1 = np.ascontiguousarray(
        wfc1.reshape(2, 4, 2, P, NFF, P)                  # (hl, i, j, p, kk, m)
        .transpose(4, 3, 1, 0, 2, 5)                      # (kk, p, i, hl, j, m)
    ).reshape(NFF, P, 4 * 2 * 2 * P)

    # wfc2 [16, 2, P, 2*1024]: per (jj, hl): (pair i, colC)
    w2hi, w2lo = _q8_hilo(w_fc2.astype(f32) * SW)         # [FF, C]
    wfc2 = np.stack([w2hi, w2lo], axis=0)                 # (hl, FF, C)
    wfc2 = np.ascontiguousarray(
        wfc2.reshape(2, 16, 2, P, C)                      # (hl, jj, i, p, colC)
        .transpose(1, 0, 3, 2, 4)                         # (jj, hl, p, i, colC)
    ).reshape(16, 2, P, 2 * C)

    shared = {
        "wqk8": wqk8,
        "wv8": wv8,
        "bqk_t": bqk_t.astype(f32),
        "wproj8": wproj8,
        "bprojrow": bproj_eff.reshape(1, C).astype(bf16),
        "wfc1": wfc1,
        "bfc_t": bfc_t.astype(f32),
        "wfc2": wfc2,
        "bfc2row": b_fc2.astype(f32).reshape(1, C).astype(bf16),
    }
    per_core = []
    t_idx = np.arange(T)
    lnSS = np.float32(np.log(SS))
    for b in range(B):
        mask = np.where(t_idx < int(seq_ls[b]), lnSS, MASK_VAL).astype(f32)
        per_core.append({
            "x": np.ascontiguousarray(x[b]).astype(f32),
            "mask_cols": mask.reshape(NT, P).T.copy(),   # [P, NT]
        })
    return shared, per_core


# --------------------------------------------------------------------------
# kernel build (single NeuronCore program, SPMD across 8 cores)
# --------------------------------------------------------------------------
def _build_nc(phases=99, repeat=1):
    nc = bacc.Bacc("TRN2", target_bir_lowering=False, debug=False,
                   num_devices=N_CORES)

    x_d = nc.dram_tensor("x", [T, C], F32, kind="ExternalInput").ap()
    mask_cols_d = nc.dram_tensor("mask_cols", [P, NT], F32,
                                 kind="ExternalInput").ap()
    wqk8_d = nc.dram_tensor("wqk8", [16, P, 8 * P], FP8,
                            kind="ExternalInput").ap()
    wv8_d = nc.dram_tensor("wv8", [2, P, 8 * 512], FP8,
                           kind="ExternalInput").ap()
    bqk_t_d = nc.dram_tensor("bqk_t", [P, 16], F32, kind="ExternalInput").ap()
    wproj8_d = nc.dram_tensor("wproj8", [2, P, 8 * 512], FP8,
                              kind="ExternalInput").ap()
    bprojrow_d = nc.dram_tensor("bprojrow", [1, C], BF16,
                                kind="ExternalInput").ap()
    wfc1_d = nc.dram_tensor("wfc1", [NFF, P, 16 * P], FP8,
                            kind="ExternalInput").ap()
    bfc_t_d = nc.dram_tensor("bfc_t", [P, NFF], F32, kind="ExternalInput").ap()
    wfc2_d = nc.dram_tensor("wfc2", [16, 2, P, 2 * C], FP8,
                            kind="ExternalInput").ap()
    bfc2row_d = nc.dram_tensor("bfc2row", [1, C], BF16,
                               kind="ExternalInput").ap()
    out_d = nc.dram_tensor("out", [T, C], F32, kind="ExternalOutput").ap()

    # DRAM access-pattern views
    x_v = x_d.rearrange("(i p) c -> p i c", p=P)          # [P, NT, C]
    out_v = out_d.rearrange("(i p) c -> p i c", p=P)
    wqk8_v = wqk8_d.rearrange("m p (i j c) -> m p i j c", i=4, j=2)
    wv8_v = wv8_d.rearrange("n p (i j c) -> n p i j c", i=4, j=2)
    wproj8_v = wproj8_d.rearrange("n p (i j c) -> n p i j c", i=4, j=2)
    wfc1_v = wfc1_d.rearrange("k p (i h j c) -> k p i h j c", i=4, h=2, j=2)
    wfc2_v = wfc2_d.rearrange("k h p (i c) -> k h p i c", i=2)

    with tile.TileContext(nc) as tc:
        with (
            tc.tile_pool(name="persist", bufs=1) as pp,
            tc.tile_pool(name="qpool", bufs=2) as qpool,
            tc.tile_pool(name="kpool", bufs=2) as kpool,
            tc.tile_pool(name="stpool", bufs=3) as stpool,
            tc.tile_pool(name="sinvb", bufs=2) as sinvbp,
            tc.tile_pool(name="small", bufs=4) as smallp,
            tc.tile_pool(name="wslab", bufs=3) as wslabp,
            tc.tile_pool(name="wrhs", bufs=2) as wrhsp,
            tc.tile_pool(name="wfc2p", bufs=4) as wfc2p,
            tc.tile_pool(name="xntok", bufs=2) as xntokp,
            tc.tile_pool(name="genps", bufs=2, space="PSUM") as genps,
            tc.tile_pool(name="qkps", bufs=4, space="PSUM") as qkps,
            tc.tile_pool(name="avps", bufs=2, space="PSUM") as avps,
        ):
            try:
                for _rep in range(repeat):
                    # ---- persistent tiles ----
                    x_sb = pp.tile([P, NT, C], F32, tag="x")            # 32KB
                    xnT8 = pp.tile([P, NCD, T], FP8, tag="xnT8")        # 8KB
                    v8 = pp.tile([P, NT, 16 * 80], FP8, tag="v8")       # 10KB
                    yT8 = pp.tile([P, NCD, T], FP8, tag="yT8")          # 8KB
                    h2T8 = pp.tile([P, NFF, T], FP8, tag="h2T8")        # 32KB
                    ident_b = pp.tile([P, P], BF16, tag="idb")
                    bproj_b = pp.tile([P, C], BF16, tag="bprojb")
                    bfc2_b = pp.tile([P, C], BF16, tag="bfc2b")
                    bqk_t = pp.tile([P, 16], F32, tag="bqkt")
                    mask_cols = pp.tile([P, NT], F32, tag="maskc")
                    bfc_t = pp.tile([P, NFF], F32, tag="bfct")

                    make_identity(nc, ident_b)
                    nc.sync.dma_start(bqk_t[:], bqk_t_d)
                    nc.sync.dma_start(mask_cols[:], mask_cols_d)
                    nc.sync.dma_start(bfc_t[:], bfc_t_d)
                    nc.sync.dma_start(bproj_b[0:1, :], bprojrow_d)
                    nc.gpsimd.partition_broadcast(bproj_b[:], bproj_b[0:1, :])
                    nc.sync.dma_start(bfc2_b[0:1, :], bfc2row_d)
                    nc.gpsimd.partition_broadcast(bfc2_b[:], bfc2_b[0:1, :])

                    # ---- load x (per-tile, so LN1 pipelines behind the DMA) ----
                    for i in range(NT):
                        nc.sync.dma_start(x_sb[:, i, :], x_v[:, i, :])

                    # ---- LayerNorm: token-major stats, xn scaled xSX,
                    #      transpose, evict fp8 feature-major dstT8 ----
                    def layernorm_to_T8(dstT8):
                        for i in range(NT):
                            xi = x_sb[:, i, :]
                            stats6 = smallp.tile([P, 2, 6], F32, tag="stats6")
                            nc.vector.bn_stats(stats6[:, 0, :], xi[:, 0:512])
                            nc.vector.bn_stats(stats6[:, 1, :], xi[:, 512:1024])
                            mv = smallp.tile([P, 2], F32, tag="mv")
                            nc.vector.bn_aggr(mv[:], stats6.rearrange("p a b -> p (a b)"))
                            rstd = smallp.tile([P, 1], F32, tag="rstd")
                            nc.vector.tensor_scalar_add(rstd[:], mv[:, 1:2], EPS)
                            nc.scalar.sqrt(rstd[:], rstd[:])
                            nc.vector.reciprocal(rstd[:], rstd[:])
                            rstd16 = smallp.tile([P, 1], F32, tag="rstd16")
                            nc.vector.tensor_scalar_mul(rstd16[:], rstd[:], SX)
                            negmr = smallp.tile([P, 1], F32, tag="negmr")
                            nc.vector.scalar_tensor_tensor(
                                negmr[:], mv[:, 0:1], -1.0, rstd16[:],
                                op0=OP.mult, op1=OP.mult)
                            xn = xntokp.tile([P, C], BF16, tag="xntok")
                            nc.scalar.activation(xn[:], xi, AF.Identity,
                                                 bias=negmr[:], scale=rstd16[:])
                            # transpose [P(t),C] -> feature-major dstT8[:, c, t]
                            for g in range(2):
                                ps = qkps.tile([P, 512], BF16, tag="qk")
                                for j in range(4):
                                    cc = 4 * g + j
                                    nc.tensor.matmul(
                                        ps[:, j * P:(j + 1) * P],
                                        xn[:, cc * P:(cc + 1) * P],
                                        ident_b[:], is_transpose=True,
                                        start=True, stop=True)
                                nc.vector.tensor_copy(
                                    dstT8[:, 4 * g:4 * g + 4, i * P:(i + 1) * P],
                                    ps.rearrange("p (a b) -> p a b", b=P))

                    layernorm_to_T8(xnT8)

                    # ---- V = xn @ wv (fp8 DR, x-stationary, key-major out;
                    # per head: 64 dims + ones(=SV) col at slot 64/144) ----
                    if phases < 2:
                        raise _PhaseDone()
                    v_view = v8.rearrange("p i (hh e) -> p i hh e", e=80)
                    nc.gpsimd.memset(v_view[:, :, :, HD:HD + 1], SV)
                    for n in range(2):
                        slab = wrhsp.tile([P, 4, 2, 512], FP8, tag="wrhs")
                        nc.sync.dma_start(
                            slab.rearrange("p i j e -> p (i j e)"), wv8_v[n])
                        for mt in range(NT):
                            ps = genps.tile([P, 512], F32, tag="gen")
                            for i in range(4):
                                nc.tensor.matmul(
                                    ps[:], xnT8[:, 2 * i:2 * i + 2,
                                                mt * P:(mt + 1) * P],
                                    slab[:, i], start=(i == 0), stop=(i == 3),
                                    perf_mode=DR)
                            # ps cols: 4 head-pairs x (even 64 | odd 64)
                            psv = ps.rearrange("p (pr two e) -> p pr two e",
                                               two=2, e=HD)
                            nc.vector.tensor_scalar_mul(
                                v_view[:, mt, 8 * n:8 * n + 8:2, 0:HD],
                                psv[:, :, 0, :], S_V_EV)
                            nc.vector.tensor_scalar_mul(
                                v_view[:, mt, 8 * n + 1:8 * n + 8:2, 0:HD],
                                psv[:, :, 1, :], S_V_EV)

                    if phases < 3:
                        raise _PhaseDone()
                    # ---- attention ----
                    for m in range(NH // 2):  # head pairs (2m, 2m+1)
                        q_sb = qpool.tile([P, T], BF16, tag="q", name=f"q_{m}")
                        k_sb = kpool.tile([P, T], BF16, tag="k", name=f"k_{m}")
                        for which, mm in ((0, m), (1, m + 8)):  # 0=q, 1=k
                            slab = wslabp.tile([P, 4, 2, P], FP8, tag="wslab",
                                               name=f"wqk_{m}_{which}")
                            nc.sync.dma_start(
                                slab.rearrange("p i j e -> p (i j e)"),
                                wqk8_v[mm])
                            dst = q_sb if which == 0 else k_sb
                            for n in range(2):
                                ps = genps.tile([P, 512], F32, tag="gen")
                                for i in range(4):
                                    nc.tensor.matmul(
                                        ps[:], slab[:, i],
                                        xnT8[:, 2 * i:2 * i + 2,
                                             n * 512:(n + 1) * 512],
                                        start=(i == 0), stop=(i == 3),
                                        perf_mode=DR)
                                nc.scalar.activation(
                                    dst[:, n * 512:(n + 1) * 512], ps[:],
                                    AF.Identity, bias=bqk_t[:, mm:mm + 1],
                                    scale=S_QKV_EV)

                        # --- QK^T row-tiled (heads at partitions 0-63/64-127,
                        # interleaved issue) + exp with mask+ln(SS) bias ---
                        st8s = []
                        for hh in range(2):
                            st8s.append(stpool.tile([P, NT, T], FP8, tag="st",
                                                    name=f"st_{m}_{hh}"))
                        for kt in range(NT):
                            pss = []
                            for n in range(2):
                                for hh in range(2):
                                    hr = slice(hh * 64, hh * 64 + 64)
                                    ps = qkps.tile([P, 512], F32, tag="qk")
                                    nc.tensor.matmul(
                                        ps[:], k_sb[hr, kt * P:(kt + 1) * P],
                                        q_sb[hr, n * 512:(n + 1) * 512],
                                        start=True, stop=True)
                                    pss.append((hh, n, ps))
                            for hh, n, ps in pss:
                                nc.scalar.activation(
                                    st8s[hh][:, kt, n * 512:(n + 1) * 512],
                                    ps[:], AF.Exp,
                                    bias=mask_cols[:, kt:kt + 1])

                        # --- AV fp8-DR over key-tile pairs; psum row 64 =
                        # softmax sums (ones=SV col in v8) ---
                        for hh in range(2):
                            h = 2 * m + hh
                            hr = slice(hh * 64, hh * 64 + 64)
                            voff = 80 * h
                            for n in range(2):
                                ps_y = avps.tile([P, 512], F32, tag="av")
                                for j in range(4):
                                    nc.tensor.matmul(
                                        ps_y[0:65, :],
                                        v8[:, 2 * j:2 * j + 2, voff:voff + 65],
                                        st8s[hh][:, 2 * j:2 * j + 2,
                                                 n * 512:(n + 1) * 512],
                                        start=(j == 0), stop=(j == 3),
                                        perf_mode=DR)
                                sinv_row = smallp.tile([1, 512], F32,
                                                       tag="sinvrow")
                                nc.vector.reciprocal(sinv_row[:],
                                                     ps_y[64:65, :])
                                sinv_b = sinvbp.tile([64, 512], F32,
                                                     tag="sinvb")
                                nc.gpsimd.partition_broadcast(
                                    sinv_b[:], sinv_row[0:1, :])
                                nc.vector.scalar_tensor_tensor(
                                    yT8[hr, m, n * 512:(n + 1) * 512],
                                    ps_y[0:64, :], SY, sinv_b[:],
                                    op0=OP.mult, op1=OP.mult)

                    if phases < 4:
                        raise _PhaseDone()
                    # ---- residual prep: x += bproj_row ----
                    for i in range(NT):
                        nc.vector.tensor_tensor(x_sb[:, i, :], x_sb[:, i, :],
                                                bproj_b[:], OP.add)

                    # ---- proj: x1 = x + y @ wproj (fp8 DR, y-stationary) ----
                    slabs = []
                    for n in range(2):
                        slab = wrhsp.tile([P, 4, 2, 512], FP8, tag="wrhs",
                                          name=f"wproj_{n}")
                        nc.sync.dma_start(
                            slab.rearrange("p i j e -> p (i j e)"), wproj8_v[n])
                        slabs.append(slab)
                    for mt in range(NT):
                        for n in range(2):
                            ps = genps.tile([P, 512], F32, tag="gen")
                            for i in range(4):
                                nc.tensor.matmul(
                                    ps[:], yT8[:, 2 * i:2 * i + 2,
                                               mt * P:(mt + 1) * P],
                                    slabs[n][:, i], start=(i == 0),
                                    stop=(i == 3), perf_mode=DR)
                            nc.vector.scalar_tensor_tensor(
                                x_sb[:, mt, n * 512:(n + 1) * 512], ps[:],
                                S_PROJ_EV, x_sb[:, mt, n * 512:(n + 1) * 512],
                                op0=OP.mult, op1=OP.add)

                    if phases < 5:
                        raise _PhaseDone()
                    # ---- LN2 -> xn2T8 (reuse xnT8 tile) ----
                    layernorm_to_T8(xnT8)

                    # ---- residual prep 2: x1 += bfc2_row ----
                    for i in range(NT):
                        nc.vector.tensor_tensor(x_sb[:, i, :], x_sb[:, i, :],
                                                bfc2_b[:], OP.add)

                    if phases < 6:
                        raise _PhaseDone()
                    # ---- FC1 (fp8 DR, weight-stationary, both token halves
                    # per stationary; hi+lo accumulate in one psum group) ----
                    for kk in range(NFF):
                        slab = wslabp.tile([P, 4, 2, 2, P], FP8, tag="wfc1")
                        nc.sync.dma_start(
                            slab.rearrange("p i h j e -> p (i h j e)"),
                            wfc1_v[kk])
                        pst = [qkps.tile([P, 512], F32, tag="qk",
                                         name=f"fc1_{kk}_{th}")
                               for th in range(2)]
                        for i in range(4):
                            for hl in range(2):
                                for th in range(2):
                                    nc.tensor.matmul(
                                        pst[th][:], slab[:, i, hl],
                                        xnT8[:, 2 * i:2 * i + 2,
                                             th * 512:(th + 1) * 512],
                                        start=(i == 0 and hl == 0),
                                        stop=(i == 3 and hl == 1),
                                        perf_mode=DR, skip_group_check=True)
                        for th in range(2):
                            nc.scalar.activation(
                                h2T8[:, kk, th * 512:(th + 1) * 512],
                                pst[th][:], AF.Relu,
                                bias=bfc_t[:, kk:kk + 1], scale=S_FC1_EV)

                    # ---- FC2 (fp8 DR, h2-stationary, both column halves per
                    # stationary; hi+lo in one psum group), per token half ----
                    for th in range(2):
                        pw = [qkps.tile([P, 512], F32, tag="qk",
                                        name=f"fc2ps_{th}_{jj}")
                              for jj in range(4)] + \
                             [avps.tile([P, 512], F32, tag="av",
                                        name=f"fc2ps_{th}_{4 + jj}")
                              for jj in range(2)] + \
                             [genps.tile([P, 512], F32, tag="gen",
                                         name=f"fc2ps_{th}_{6 + jj}")
                              for jj in range(2)]
                        # accumulator (ch, mt) -> pw[2*ch + ...]: flat index
                        pss = [[pw[4 * ch + mt] for mt in range(4)]
                               for ch in range(2)]
                        for jj in range(16):
                            for hl in range(2):
                                rhs = wfc2p.tile([P, 2, C], FP8, tag="wfc2")
                                nc.sync.dma_start(
                                    rhs.rearrange("p i e -> p (i e)"),
                                    wfc2_v[jj, hl])
                                for mt in range(4):
                                    tok = (4 * th + mt) * P
                                    for ch in range(2):
                                        nc.tensor.matmul(
                                            pss[ch][mt][:],
                                            h2T8[:, 2 * jj:2 * jj + 2,
                                                 tok:tok + P],
                                            rhs[:, :, ch * 512:(ch + 1) * 512],
                                            start=(jj == 0 and hl == 0),
                                            stop=(jj == 15 and hl == 1),
                                            perf_mode=DR,
                                            skip_group_check=True)
                        for ch in range(2):
                            for mt in range(4):
                                i = 4 * th + mt
                                nc.vector.scalar_tensor_tensor(
                                    x_sb[:, i, ch * 512:(ch + 1) * 512],
                                    pss[ch][mt][:], S_FC2_EV,
                                    x_sb[:, i, ch * 512:(ch + 1) * 512],
                                    op0=OP.mult, op1=OP.add)
                        for mt in range(4):
                            i = 4 * th + mt
                            nc.sync.dma_start(out_v[:, i, :], x_sb[:, i, :])
            except _PhaseDone:
                pass

    nc.compile()
    return nc


class _PhaseDone(Exception):
    pass


_NC_CACHE = None


def _get_nc():
    global _NC_CACHE
    if _NC_CACHE is None:
        _NC_CACHE = _build_nc()
    return _NC_CACHE


def _run(inputs, trace=False, **kwargs):
    shared, per_core = _host_prep(**inputs)
    nc = _get_nc()
    in_maps = [{**shared, **pc} for pc in per_core]
    res = run_bass_kernel_spmd(nc, in_maps, core_ids=list(range(N_CORES)),
                               trace=trace, **kwargs)
    out = np.stack([res.results[i]["out"] for i in range(N_CORES)], axis=0)
    return out.astype(np.float32), res


def kernel(**inputs):
    return _run(inputs)[0]


# revision 19
# speedup vs baseline: 1.2323x; 1.2323x over previous
"""Trainium2 Bass kernel for a dense transformer block (pre-LN, masked attention).

Sharding: data-parallel over batch B=8 across the 8 NeuronCores — each core
processes one full batch element [T=1024, C=1024]; weights are replicated.
No collectives needed.

Per-core dataflow (single NeuronCore), fp8-heavy:
  - LN1 token-major stats; normalized xn scaled x16 (bf16), PE-transposed,
    evicted to fp8 xnT8 [C, T] (e4m3, DVE copy).
  - QKV in fp8 DoubleRow (contraction pairs of 128-blocks): weights
    pre-quantized x512 on host.  Q/K evicted bf16 in true units (DVE, bias
    broadcast along free dim).  V evicted fp8 x32 key-major with a ones(=32)
    column per head so AV's psum row 64 gives the softmax sums.
  - QK^T bf16 (contraction = 64 head dims): the two heads of a pair occupy
    partitions 0-63 / 64-127, issued interleaved so their matmuls run
    concurrently in different row-groups of the PE array (tile_position
    auto-derived).  exp on ScalarE ([128,1024] ops) with key-padding mask +
    ln(8) bias, evicted straight to fp8 st8 (x8).
  - AV in fp8 DoubleRow over key-tile pairs; normalization fused into the
    yT8 eviction (x512, DVE scalar_tensor_tensor with broadcast 1/rowsum).
  - proj in fp8 DoubleRow, eviction fused with residual add (DVE).
  - LN2 -> xn2T8 (same path as LN1).
  - FC1/FC2 in fp8 DoubleRow with same-scale hi/lo weight split (w = fp8(w)
    + fp8(w - fp8(w)): residual lands in e4m3 denormals, so weights carry
    ~bf16 accuracy; both halves accumulate in one PSUM group).  FC1 is
    weight-stationary (each stationary reused for both token halves); FC2
    token-major (each h2T8 stationary reused for both output-column halves).
  - PSUM: one pool of 3 x [P,1024] (2-bank) tiles + one of 2 x [P,512];
    evictions run on [P,1024] at once to halve per-op overheads.
"""

import os
import sys
import numpy as np
import ml_dtypes

for _p in ("/opt/trn_rl_repo", "/opt/pypackages"):
    if os.path.isdir(_p) and _p not in sys.path:
        sys.path.append(_p)

import concourse.bass as bass
import concourse.mybir as mybir
import concourse.tile as tile
from concourse import bacc
from concourse.bass_utils import run_bass_kernel_spmd
from concourse.masks import make_identity

P = 128
B, T, C = 8, 1024, 1024
NH, HD = 16, 64
FF = 4 * C
EPS = 1e-5
NT = T // P      # 8 token tiles
NCD = C // P     # 8 feature tiles
NFF = FF // P    # 32 ff tiles
N_CORES = 8
MASK_VAL = -30000.0

F32 = mybir.dt.float32
BF16 = mybir.dt.bfloat16
FP8 = mybir.dt.float8e4
AF = mybir.ActivationFunctionType
OP = mybir.AluOpType
DR = mybir.MatmulPerfMode.DoubleRow

bf16 = ml_dtypes.bfloat16
E4 = ml_dtypes.float8_e4m3

# power-of-2 quantization scales
SX = 16.0     # normalized activations (xn, xn2)
SW = 512.0    # weights
SV = 32.0     # v
SS = 8.0      # st = exp(logits)
SY = 512.0    # y (attention out)
SH = 32.0     # h2 scale
S_QKV_EV = 1.0 / (SX * SW)          # q/k eviction: psum -> true units
S_V_EV = SV / (SX * SW)             # v eviction
S_PROJ_EV = 1.0 / (SY * SW)         # proj eviction
S_FC1_EV = SH / (SX * SW)           # fc1 eviction: h2*32 = relu(ps/256+32b)
S_FC2_EV = 1.0 / (SH * SW)          # fc2 eviction


def _q8(a):
    return np.clip(a, -240, 240).astype(E4)


def _q8_hilo(a):
    """same-scale hi/lo split: returns (hi, lo) fp8 arrays; hi+lo ~ a."""
    hi = _q8(a)
    lo = _q8(a - hi.astype(np.float32))
    return hi, lo


# --------------------------------------------------------------------------
# host-side preparation: fold LN gains/biases into weights, quantize to fp8
# --------------------------------------------------------------------------
def _host_prep(x, seq_ls, ln1_g, ln1_b, w_qkv, b_qkv, w_proj, b_proj,
               ln2_g, ln2_b, w_fc, b_fc, w_fc2, b_fc2):
    f32 = np.float32
    ln1_g, ln1_b = ln1_g.astype(f32), ln1_b.astype(f32)
    w_qkv = w_qkv.astype(f32)

    wqkv_eff = ln1_g[:, None] * w_qkv                     # [C, 3C]
    bqkv_eff = ln1_b @ w_qkv + b_qkv.astype(f32)          # [3C]
    scale = np.float32(1.0 / np.sqrt(HD))
    wq = wqkv_eff[:, :C] * scale
    bq = bqkv_eff[:C] * scale
    wk = wqkv_eff[:, C:2 * C]
    bk = bqkv_eff[C:2 * C]
    wv = wqkv_eff[:, 2 * C:]
    bv = bqkv_eff[2 * C:]

    bproj_eff = bv @ w_proj.astype(f32) + b_proj.astype(f32)   # [C]

    wfc_eff = ln2_g.astype(f32)[:, None] * w_fc.astype(f32)    # [C, FF]
    bfc_eff = ln2_b.astype(f32) @ w_fc.astype(f32) + b_fc.astype(f32)

    wqk = np.concatenate([wq, wk], axis=1)                # [C, 2C]
    bqk_t = np.concatenate([bq, bk]).reshape(16, P).T.copy()   # [P, 16]
    bfc_t = (bfc_eff * SH).reshape(NFF, P).T.copy()       # [P, 32] (x32)

    # --- fp8 weight layouts ---
    # wqk8 [16, P, 4*2*128]: per head-pair column block mm: (kop, pair, col)
    wqk_s = _q8(wqk * SW)                                 # [C, 2C]
    wqk8 = np.ascontiguousarray(
        wqk_s.reshape(4, 2, P, 16, P)                     # (i, j, p, mm, m)
        .transpose(3, 2, 0, 1, 4)                         # (mm, p, i, j, m)
    ).reshape(16, P, 4 * 2 * P)

    # wv8 [2, P, 4*2*512]: slab n covers output cols n*512..: (kop, pair, col)
    wv_s = _q8(wv * SW)                                   # [C, C]
    wv8 = np.ascontiguousarray(
        wv_s.reshape(4, 2, P, 2, 512)                     # (i, j, p, n, e)
        .transpose(3, 2, 0, 1, 4)                         # (n, p, i, j, e)
    ).reshape(2, P, 4 * 2 * 512)

    # wproj8 [2, P, 4*2*512]
    wp_s = _q8(w_proj.astype(f32) * SW)
    wproj8 = np.ascontiguousarray(
        wp_s.reshape(4, 2, P, 2, 512).transpose(3, 2, 0, 1, 4)
    ).reshape(2, P, 4 * 2 * 512)

    # wfc1 [32, P, 4*2*2*128]: per kk: (kop i, hl, pair j, col m)
    whi, wlo = _q8_hilo(wfc_eff * SW)                     # [C, FF] each
    wfc1 = np.stack([whi, wlo], axis=0)                   # (hl, C, FF)
    wfc1 = np.ascontiguousarray(
        wfc1.reshape(2, 4, 2, P, NFF, P)                  # (hl, i, j, p, kk, m)
        .transpose(4, 3, 1, 0, 2, 5)                      # (kk, p, i, hl, j, m)
    ).reshape(NFF, P, 4 * 2 * 2 * P)

    # wfc2 [16, 2, P, 2*1024]: per (jj, hl): (pair i, colC)
    w2hi, w2lo = _q8_hilo(w_fc2.astype(f32) * SW)         # [FF, C]
    wfc2 = np.stack([w2hi, w2lo], axis=0)                 # (hl, FF, C)
    wfc2 = np.ascontiguousarray(
        wfc2.reshape(2, 16, 2, P, C)                      # (hl, jj, i, p, colC)
        .transpose(1, 0, 3, 2, 4)                         # (jj, hl, p, i, colC)
    ).reshape(16, 2, P, 2 * C)

    shared = {
        "wqk8": wqk8,
        "wv8": wv8,
        "bqk_t": bqk_t.astype(f32),
        "wproj8": wproj8,
        "bprojrow": bproj_eff.reshape(1, C).astype(bf16),
        "wfc1": wfc1,
        "bfc_t": bfc_t.astype(f32),
        "wfc2": wfc2,
        "bfc2row": b_fc2.astype(f32).reshape(1, C).astype(bf16),
    }
    per_core = []
    t_idx = np.arange(T)
    lnSS = np.float32(np.log(SS))
    for b in range(B):
        mask = np.where(t_idx < int(seq_ls[b]), lnSS, MASK_VAL).astype(f32)
        per_core.append({
            "x": np.ascontiguousarray(x[b]).astype(f32),
            "mask_cols": mask.reshape(NT, P).T.copy(),   # [P, NT]
        })
    return shared, per_core


# --------------------------------------------------------------------------
# kernel build (single NeuronCore program, SPMD across 8 cores)
# --------------------------------------------------------------------------
def _build_nc(phases=99, repeat=1):
    nc = bacc.Bacc("TRN2", target_bir_lowering=False, debug=False,
                   num_devices=N_CORES)

    x_d = nc.dram_tensor("x", [T, C], F32, kind="ExternalInput").ap()
    mask_cols_d = nc.dram_tensor("mask_cols", [P, NT], F32,
                                 kind="ExternalInput").ap()
    wqk8_d = nc.dram_tensor("wqk8", [16, P, 8 * P], FP8,
                            kind="ExternalInput").ap()
    wv8_d = nc.dram_tensor("wv8", [2, P, 8 * 512], FP8,
                           kind="ExternalInput").ap()
    bqk_t_d = nc.dram_tensor("bqk_t", [P, 16], F32, kind="ExternalInput").ap()
    wproj8_d = nc.dram_tensor("wproj8", [2, P, 8 * 512], FP8,
                              kind="ExternalInput").ap()
    bprojrow_d = nc.dram_tensor("bprojrow", [1, C], BF16,
                                kind="ExternalInput").ap()
    wfc1_d = nc.dram_tensor("wfc1", [NFF, P, 16 * P], FP8,
                            kind="ExternalInput").ap()
    bfc_t_d = nc.dram_tensor("bfc_t", [P, NFF], F32, kind="ExternalInput").ap()
    wfc2_d = nc.dram_tensor("wfc2", [16, 2, P, 2 * C], FP8,
                            kind="ExternalInput").ap()
    bfc2row_d = nc.dram_tensor("bfc2row", [1, C], BF16,
                               kind="ExternalInput").ap()
    out_d = nc.dram_tensor("out", [T, C], F32, kind="ExternalOutput").ap()

    # DRAM access-pattern views
    x_v = x_d.rearrange("(i p) c -> p i c", p=P)          # [P, NT, C]
    out_v = out_d.rearrange("(i p) c -> p i c", p=P)
    wqk8_v = wqk8_d.rearrange("m p (i j c) -> m p i j c", i=4, j=2)
    wv8_v = wv8_d.rearrange("n p (i j c) -> n p i j c", i=4, j=2)
    wproj8_v = wproj8_d.rearrange("n p (i j c) -> n p i j c", i=4, j=2)
    wfc1_v = wfc1_d.rearrange("k p (i h j c) -> k p i h j c", i=4, h=2, j=2)
    wfc2_v = wfc2_d.rearrange("k h p (i c) -> k h p i c", i=2)

    with tile.TileContext(nc) as tc:
        with (
            tc.tile_pool(name="persist", bufs=1) as pp,
            tc.tile_pool(name="qpool", bufs=2) as qpool,
            tc.tile_pool(name="kpool", bufs=2) as kpool,
            tc.tile_pool(name="stpool", bufs=4) as stpool,
            tc.tile_pool(name="sinvb", bufs=2) as sinvbp,
            tc.tile_pool(name="small", bufs=4) as smallp,
            tc.tile_pool(name="wslab", bufs=4) as wslabp,
            tc.tile_pool(name="wrhs", bufs=2) as wrhsp,
            tc.tile_pool(name="wfc2p", bufs=4) as wfc2p,
            tc.tile_pool(name="xntok", bufs=2) as xntokp,
            tc.tile_pool(name="bigps", bufs=2, space="PSUM") as bigps,
            tc.tile_pool(name="avps", bufs=2, space="PSUM") as avps,
        ):
            try:
                for _rep in range(repeat):
                    # ---- persistent tiles ----
                    x_sb = pp.tile([P, NT, C], F32, tag="x")            # 32KB
                    xnT8 = pp.tile([P, NCD, T], FP8, tag="xnT8")        # 8KB
                    v8 = pp.tile([P, NT, 16 * P], FP8, tag="v8")        # 16KB
                    yT8 = pp.tile([P, NCD, T], FP8, tag="yT8")          # 8KB
                    h2T8 = pp.tile([P, NFF, T], FP8, tag="h2T8")        # 32KB
                    ident_f = pp.tile([P, P], F32, tag="idf")
                    bproj_b = pp.tile([P, C], BF16, tag="bprojb")
                    bfc2_b = pp.tile([P, C], BF16, tag="bfc2b")
                    bqk_t = pp.tile([P, 16], F32, tag="bqkt")
                    mask_cols = pp.tile([P, NT], F32, tag="maskc")
                    bfc_t = pp.tile([P, NFF], F32, tag="bfct")

                    make_identity(nc, ident_f)
                    nc.sync.dma_start(bqk_t[:], bqk_t_d)
                    nc.sync.dma_start(mask_cols[:], mask_cols_d)
                    nc.sync.dma_start(bfc_t[:], bfc_t_d)
                    nc.sync.dma_start(bproj_b[0:1, :], bprojrow_d)
                    nc.gpsimd.partition_broadcast(bproj_b[:], bproj_b[0:1, :])
                    nc.sync.dma_start(bfc2_b[0:1, :], bfc2row_d)
                    nc.gpsimd.partition_broadcast(bfc2_b[:], bfc2_b[0:1, :])

                    # ---- load x (per-tile, so LN1 pipelines behind the DMA) ----
                    for i in range(NT):
                        nc.sync.dma_start(x_sb[:, i, :], x_v[:, i, :])

                    # ---- LayerNorm: token-major stats, xn scaled xSX,
                    #      transpose, evict fp8 feature-major dstT8 ----
                    def layernorm_to_T8(dstT8):
                        for i in range(NT):
                            xi = x_sb[:, i, :]
                            stats6 = smallp.tile([P, 2, 6], F32, tag="stats6")
                            nc.vector.bn_stats(stats6[:, 0, :], xi[:, 0:512])
                            nc.vector.bn_stats(stats6[:, 1, :], xi[:, 512:1024])
                            mv = smallp.tile([P, 2], F32, tag="mv")
                            nc.vector.bn_aggr(mv[:], stats6.rearrange("p a b -> p (a b)"))
                            rstd = smallp.tile([P, 1], F32, tag="rstd")
                            nc.vector.tensor_scalar_add(rstd[:], mv[:, 1:2], EPS)
                            nc.scalar.sqrt(rstd[:], rstd[:])
                            nc.vector.reciprocal(rstd[:], rstd[:])
                            rstd16 = smallp.tile([P, 1], F32, tag="rstd16")
                            nc.vector.tensor_scalar_mul(rstd16[:], rstd[:], SX)
                            negmr = smallp.tile([P, 1], F32, tag="negmr")
                            nc.vector.scalar_tensor_tensor(
                                negmr[:], mv[:, 0:1], -1.0, rstd16[:],
                                op0=OP.mult, op1=OP.mult)
                            xn = xntokp.tile([P, C], F32, tag="xntok")
                            nc.scalar.activation(xn[:], xi, AF.Identity,
                                                 bias=negmr[:], scale=rstd16[:])
                            # transpose [P(t),C] -> feature-major dstT8[:, c, t]
                            ps = bigps.tile([P, T], F32, tag="big")
                            for j in range(NCD):
                                nc.tensor.matmul(
                                    ps[:, j * P:(j + 1) * P],
                                    xn[:, j * P:(j + 1) * P],
                                    ident_f[:], is_transpose=True,
                                    start=True, stop=True,
                                    skip_group_check=True)
                            nc.vector.tensor_copy(
                                dstT8[:, :, i * P:(i + 1) * P],
                                ps.rearrange("p (a b) -> p a b", b=P))

                    layernorm_to_T8(xnT8)

                    # ---- V = xn @ wv (fp8 DR, x-stationary, key-major out;
                    # per head: 64 dims + ones(=SV) col at slot h*80+64) ----
                    if phases < 2:
                        raise _PhaseDone()
                    v_view = v8.rearrange("p i (hh e) -> p i hh e", e=P)
                    nc.gpsimd.memset(v_view[:, :, :, HD:P], SV)
                    vslabs = []
                    for n in range(2):
                        slab = wrhsp.tile([P, 4, 2, 512], FP8, tag="wrhs",
                                          name=f"wv_{n}")
                        nc.sync.dma_start(
                            slab.rearrange("p i j e -> p (i j e)"), wv8_v[n])
                        vslabs.append(slab)
                    for mt in range(NT):
                        ps = bigps.tile([P, T], F32, tag="big")
                        for n in range(2):
                            for i in range(4):
                                nc.tensor.matmul(
                                    ps[:, n * 512:(n + 1) * 512],
                                    xnT8[:, 2 * i:2 * i + 2,
                                         mt * P:(mt + 1) * P],
                                    vslabs[n][:, i], start=(i == 0),
                                    stop=(i == 3), perf_mode=DR,
                                    skip_group_check=True)
                        # ps cols: (n, pr, parity, 64): head h = 8n+2pr+q
                        psv = ps.rearrange("p (n pr two e) -> p (n pr) two e",
                                           two=2, e=HD, n=2)
                        nc.vector.tensor_scalar_mul(
                            v_view[:, mt, 0:16:2, 0:HD], psv[:, :, 0, :],
                            S_V_EV)
                        nc.vector.tensor_scalar_mul(
                            v_view[:, mt, 1:16:2, 0:HD], psv[:, :, 1, :],
                            S_V_EV)

                    if phases < 3:
                        raise _PhaseDone()
                    # ---- attention ----
                    def qkgen(m):
                        q_sb = qpool.tile([P, T], BF16, tag="q", name=f"q_{m}")
                        k_sb = kpool.tile([P, T], BF16, tag="k", name=f"k_{m}")
                        for which, mm in ((0, m), (1, m + 8)):  # 0=q, 1=k
                            slab = wslabp.tile([P, 4, 2, P], FP8, tag="wslab",
                                               name=f"wqk_{m}_{which}")
                            nc.sync.dma_start(
                                slab.rearrange("p i j e -> p (i j e)"),
                                wqk8_v[mm])
                            dst = q_sb if which == 0 else k_sb
                            ps = bigps.tile([P, T], F32, tag="big")
                            for n in range(2):
                                for i in range(4):
                                    nc.tensor.matmul(
                                        ps[:, n * 512:(n + 1) * 512],
                                        slab[:, i],
                                        xnT8[:, 2 * i:2 * i + 2,
                                             n * 512:(n + 1) * 512],
                                        start=(i == 0), stop=(i == 3),
                                        perf_mode=DR, skip_group_check=True)
                            nc.vector.scalar_tensor_tensor(
                                dst[:], ps[:], S_QKV_EV,
                                bqk_t[:, mm:mm + 1].broadcast_to((P, T)),
                                op0=OP.mult, op1=OP.add)
                        return q_sb, k_sb

                    qk_next = qkgen(0)

                    def av_group(m_, j, st8s_, ps_ys_):
                        for hh in range(2):
                            voff = P * (2 * m_ + hh)
                            for n in range(2):
                                nc.tensor.matmul(
                                    ps_ys_[hh][:, n * 512:(n + 1) * 512],
                                    v8[:, 2 * j:2 * j + 2, voff:voff + P],
                                    st8s_[hh][:, 2 * j:2 * j + 2,
                                              n * 512:(n + 1) * 512],
                                    start=(j == 0), stop=(j == 3),
                                    perf_mode=DR, skip_group_check=True)

                    def normalize(m_, ps_ys_):
                        # sums sit in psum rows 64-127 (replicated ones cols)
                        for hh in range(2):
                            hr = slice(hh * 64, hh * 64 + 64)
                            sinv64 = sinvbp.tile([64, T], F32, tag="sinvb")
                            nc.vector.reciprocal(sinv64[:],
                                                 ps_ys_[hh][64:128, :])
                            nc.vector.scalar_tensor_tensor(
                                yT8[hr, m_, :], ps_ys_[hh][0:64, :], SY,
                                sinv64[:], op0=OP.mult, op1=OP.mult)

                    pending = None   # (m, st8s, ps_ys) with AV j=3 + norm due
                    for m in range(NH // 2):  # head pairs (2m, 2m+1)
                        q_sb, k_sb = qk_next
                        st8s = []
                        for hh in range(2):
                            st8s.append(stpool.tile([P, NT, T], FP8, tag="st",
                                                    name=f"st_{m}_{hh}"))
                        ps_ys = [avps.tile([P, T], F32, tag="av",
                                           name=f"av_{m}_{hh}")
                                 for hh in range(2)]
                        for kt in range(NT):
                            pss = []
                            for hh in range(2):
                                ps_kt = bigps.tile([P, T], F32, tag="big")
                                pss.append(ps_kt)
                            for n in range(2):
                                for hh in range(2):
                                    hr = slice(hh * 64, hh * 64 + 64)
                                    nc.tensor.matmul(
                                        pss[hh][:, n * 512:(n + 1) * 512],
                                        k_sb[hr, kt * P:(kt + 1) * P],
                                        q_sb[hr, n * 512:(n + 1) * 512],
                                        start=True, stop=True,
                                        skip_group_check=True)
                            if kt == 0 and pending is not None:
                                pm, pst8s, pps = pending
                                av_group(pm, 3, pst8s, pps)
                                normalize(pm, pps)
                                pending = None
                            for hh in range(2):
                                nc.scalar.activation(
                                    st8s[hh][:, kt, :], pss[hh][:], AF.Exp,
                                    bias=mask_cols[:, kt:kt + 1])
                            if kt == 0 and m + 1 < NH // 2:
                                # prefetch next pair's q/k during exp work
                                qk_next = qkgen(m + 1)
                            # AV for key-tile pair j once st8 of kt=2j+1 done
                            if kt % 2 == 1 and kt < 7:
                                av_group(m, kt // 2, st8s, ps_ys)
                        pending = (m, st8s, ps_ys)
                    pm, pst8s, pps = pending
                    av_group(pm, 3, pst8s, pps)
                    normalize(pm, pps)

                    if phases < 4:
                        raise _PhaseDone()
                    # ---- residual prep: x += bproj_row ----
                    for i in range(NT):
                        nc.vector.tensor_tensor(x_sb[:, i, :], x_sb[:, i, :],
                                                bproj_b[:], OP.add)

                    # ---- proj: x1 = x + y @ wproj (fp8 DR, y-stationary) ----
                    pslabs = []
                    for n in range(2):
                        slab = wrhsp.tile([P, 4, 2, 512], FP8, tag="wrhs",
                                          name=f"wproj_{n}")
                        nc.sync.dma_start(
                            slab.rearrange("p i j e -> p (i j e)"), wproj8_v[n])
                        pslabs.append(slab)
                    for mt in range(NT):
                        ps = bigps.tile([P, T], F32, tag="big")
                        for n in range(2):
                            for i in range(4):
                                nc.tensor.matmul(
                                    ps[:, n * 512:(n + 1) * 512],
                                    yT8[:, 2 * i:2 * i + 2,
                                        mt * P:(mt + 1) * P],
                                    pslabs[n][:, i], start=(i == 0),
                                    stop=(i == 3), perf_mode=DR,
                                    skip_group_check=True)
                        nc.vector.scalar_tensor_tensor(
                            x_sb[:, mt, :], ps[:], S_PROJ_EV, x_sb[:, mt, :],
                            op0=OP.mult, op1=OP.add)

                    if phases < 5:
                        raise _PhaseDone()
                    # ---- LN2 -> xn2T8 (reuse xnT8 tile) ----
                    layernorm_to_T8(xnT8)

                    # ---- residual prep 2: x1 += bfc2_row ----
                    for i in range(NT):
                        nc.vector.tensor_tensor(x_sb[:, i, :], x_sb[:, i, :],
                                                bfc2_b[:], OP.add)

                    if phases < 6:
                        raise _PhaseDone()
                    # ---- FC1 (fp8 DR, weight-stationary, both token halves
                    # per stationary; hi+lo accumulate in one psum group) ----
                    for kk in range(NFF):
                        slab = wslabp.tile([P, 4, 2, 2, P], FP8, tag="wfc1")
                        nc.sync.dma_start(
                            slab.rearrange("p i h j e -> p (i h j e)"),
                            wfc1_v[kk])
                        ps = bigps.tile([P, T], F32, tag="big")
                        for i in range(4):
                            for hl in range(2):
                                for th in range(2):
                                    nc.tensor.matmul(
                                        ps[:, th * 512:(th + 1) * 512],
                                        slab[:, i, hl],
                                        xnT8[:, 2 * i:2 * i + 2,
                                             th * 512:(th + 1) * 512],
                                        start=(i == 0 and hl == 0),
                                        stop=(i == 3 and hl == 1),
                                        perf_mode=DR, skip_group_check=True)
                        nc.scalar.activation(
                            h2T8[:, kk, :], ps[:], AF.Relu,
                            bias=bfc_t[:, kk:kk + 1], scale=S_FC1_EV)

                    # ---- FC2 (fp8 DR, h2-stationary, both column halves per
                    # stationary; hi+lo in one psum group), per token half ----
                    for th in range(2):
                        big4 = [bigps.tile([P, T], F32, tag="big",
                                           name=f"fc2ps_{th}_{jj}")
                                for jj in range(2)] + \
                               [avps.tile([P, T], F32, tag="av",
                                          name=f"fc2av_{th}_{jj}")
                                for jj in range(2)]
                        slots = [big4[0][:, 0:512], big4[0][:, 512:1024],
                                 big4[1][:, 0:512], big4[1][:, 512:1024],
                                 big4[2][:, 0:512], big4[2][:, 512:1024],
                                 big4[3][:, 0:512], big4[3][:, 512:1024]]
                        # accumulator (ch, mt) -> slots[4*ch + mt]
                        pss = [[slots[4 * ch + mt] for mt in range(4)]
                               for ch in range(2)]
                        for jj in range(16):
                            for hl in range(2):
                                rhs = wfc2p.tile([P, 2, C], FP8, tag="wfc2")
                                nc.sync.dma_start(
                                    rhs.rearrange("p i e -> p (i e)"),
                                    wfc2_v[jj, hl])
                                for mt in range(4):
                                    tok = (4 * th + mt) * P
                                    for ch in range(2):
                                        nc.tensor.matmul(
                                            pss[ch][mt],
                                            h2T8[:, 2 * jj:2 * jj + 2,
                                                 tok:tok + P],
                                            rhs[:, :, ch * 512:(ch + 1) * 512],
                                            start=(jj == 0 and hl == 0),
                                            stop=(jj == 15 and hl == 1),
                                            perf_mode=DR,
                                            skip_group_check=True)
                        for ch in range(2):
                            for mt in range(4):
                                i = 4 * th + mt
                                nc.vector.scalar_tensor_tensor(
                                    x_sb[:, i, ch * 512:(ch + 1) * 512],
                                    pss[ch][mt], S_FC2_EV,
                                    x_sb[:, i, ch * 512:(ch + 1) * 512],
                                    op0=OP.mult, op1=OP.add)
                        for mt in range(4):
                            i = 4 * th + mt
                            nc.sync.dma_start(out_v[:, i, :], x_sb[:, i, :])
            except _PhaseDone:
                pass

    nc.compile()
    return nc


class _PhaseDone(Exception):
    pass


_NC_CACHE = None


def _get_nc():
    global _NC_CACHE
    if _NC_CACHE is None:
        _NC_CACHE = _build_nc()
    return _NC_CACHE


def _run(inputs, trace=False, **kwargs):
    shared, per_core = _host_prep(**inputs)
    nc = _get_nc()
    in_maps = [{**shared, **pc} for pc in per_core]
    res = run_bass_kernel_spmd(nc, in_maps, core_ids=list(range(N_CORES)),
                               trace=trace, **kwargs)
    out = np.stack([res.results[i]["out"] for i in range(N_CORES)], axis=0)
    return out.astype(np.float32), res


def kernel(**inputs):
    return _run(inputs)[0]


# revision 24
# speedup vs baseline: 2.0374x; 1.6533x over previous
"""Trainium2 Bass kernel for a dense transformer block (pre-LN, masked attention).

Sharding: data-parallel over batch B=8 across the 8 NeuronCores — each core
processes one full batch element [T=1024, C=1024]; weights are replicated.
No collectives needed.

Per-core dataflow (single NeuronCore), fp8-heavy:
  - LN1 token-major stats; normalized xn scaled x16 (bf16), PE-transposed,
    evicted to fp8 xnT8 [C, T] (e4m3, DVE copy).
  - QKV in fp8 DoubleRow (contraction pairs of 128-blocks): weights
    pre-quantized x512 on host.  Q/K evicted bf16 in true units (DVE, bias
    broadcast along free dim).  V evicted fp8 x32 key-major with a ones(=32)
    column per head so AV's psum row 64 gives the softmax sums.
  - QK^T bf16 (contraction = 64 head dims): the two heads of a pair occupy
    partitions 0-63 / 64-127, issued interleaved so their matmuls run
    concurrently in different row-groups of the PE array (tile_position
    auto-derived).  exp on ScalarE ([128,1024] ops) with key-padding mask +
    ln(8) bias, evicted straight to fp8 st8 (x8).
  - AV in fp8 DoubleRow over key-tile pairs; normalization fused into the
    yT8 eviction (x512, DVE scalar_tensor_tensor with broadcast 1/rowsum).
  - proj in fp8 DoubleRow, eviction fused with residual add (DVE).
  - LN2 -> xn2T8 (same path as LN1).
  - FC1/FC2 in fp8 DoubleRow with same-scale hi/lo weight split (w = fp8(w)
    + fp8(w - fp8(w)): residual lands in e4m3 denormals, so weights carry
    ~bf16 accuracy; both halves accumulate in one PSUM group).  FC1 is
    weight-stationary (each stationary reused for both token halves); FC2
    token-major (each h2T8 stationary reused for both output-column halves).
  - PSUM: one pool of 3 x [P,1024] (2-bank) tiles + one of 2 x [P,512];
    evictions run on [P,1024] at once to halve per-op overheads.
"""

import os
import sys
import numpy as np
import ml_dtypes

for _p in ("/opt/trn_rl_repo", "/opt/pypackages"):
    if os.path.isdir(_p) and _p not in sys.path:
        sys.path.append(_p)

import concourse.bass as bass
import concourse.mybir as mybir
import concourse.tile as tile
from concourse import bacc
from concourse.bass_utils import run_bass_kernel_spmd
from concourse.masks import make_identity

P = 128
B, T, C = 8, 1024, 1024
NH, HD = 16, 64
FF = 4 * C
EPS = 1e-5
NT = T // P      # 8 token tiles
NCD = C // P     # 8 feature tiles
NFF = FF // P    # 32 ff tiles
N_CORES = 8
MASK_VAL = -30000.0

F32 = mybir.dt.float32
BF16 = mybir.dt.bfloat16
FP8 = mybir.dt.float8e4
AF = mybir.ActivationFunctionType
OP = mybir.AluOpType
DR = mybir.MatmulPerfMode.DoubleRow

bf16 = ml_dtypes.bfloat16
E4 = ml_dtypes.float8_e4m3

# power-of-2 quantization scales
SX = 16.0     # normalized activations (xn, xn2)
SW = 512.0    # weights
SV = 32.0     # v
SS = 8.0      # st = exp(logits)
SY = 512.0    # y (attention out)
SH = 32.0     # h2 scale
S_QKV_EV = 1.0 / (SX * SW)          # q/k eviction: psum -> true units
S_V_EV = SV / (SX * SW)             # v eviction
S_PROJ_EV = 1.0 / (SY * SW)         # proj eviction
S_FC1_EV = SH / (SX * SW)           # fc1 eviction: h2*32 = relu(ps/256+32b)
S_FC2_EV = 1.0 / (SH * SW)          # fc2 eviction


def _q8(a):
    return np.clip(a, -240, 240).astype(E4)


def _q8_hilo(a):
    """same-scale hi/lo split: returns (hi, lo) fp8 arrays; hi+lo ~ a."""
    hi = _q8(a)
    lo = _q8(a - hi.astype(np.float32))
    return hi, lo


# --------------------------------------------------------------------------
# host-side preparation: fold LN gains/biases into weights, quantize to fp8
# --------------------------------------------------------------------------
def _host_prep(x, seq_ls, ln1_g, ln1_b, w_qkv, b_qkv, w_proj, b_proj,
               ln2_g, ln2_b, w_fc, b_fc, w_fc2, b_fc2):
    f32 = np.float32
    ln1_g, ln1_b = ln1_g.astype(f32), ln1_b.astype(f32)
    w_qkv = w_qkv.astype(f32)

    wqkv_eff = ln1_g[:, None] * w_qkv                     # [C, 3C]
    bqkv_eff = ln1_b @ w_qkv + b_qkv.astype(f32)          # [3C]
    scale = np.float32(1.0 / np.sqrt(HD))
    wq = wqkv_eff[:, :C] * scale
    bq = bqkv_eff[:C] * scale
    wk = wqkv_eff[:, C:2 * C]
    bk = bqkv_eff[C:2 * C]
    wv = wqkv_eff[:, 2 * C:]
    bv = bqkv_eff[2 * C:]

    bproj_eff = bv @ w_proj.astype(f32) + b_proj.astype(f32)   # [C]

    wfc_eff = ln2_g.astype(f32)[:, None] * w_fc.astype(f32)    # [C, FF]
    bfc_eff = ln2_b.astype(f32) @ w_fc.astype(f32) + b_fc.astype(f32)

    wqk = np.concatenate([wq, wk], axis=1)                # [C, 2C]
    bqk_t = np.concatenate([bq, bk]).reshape(16, P).T.copy()   # [P, 16]
    bfc_t = (bfc_eff * SH).reshape(NFF, P).T.copy()       # [P, 32] (x32)

    # --- fp8 weight layouts ---
    # wqk8 [16, P, 4*2*128]: per head-pair column block mm: (kop, pair, col)
    wqk_s = _q8(wqk * SW)                                 # [C, 2C]
    wqk8 = np.ascontiguousarray(
        wqk_s.reshape(4, 2, P, 16, P)                     # (i, j, p, mm, m)
        .transpose(3, 2, 0, 1, 4)                         # (mm, p, i, j, m)
    ).reshape(16, P, 4 * 2 * P)

    # wv8 [2, P, 4*2*512]: slab n covers output cols n*512..: (kop, pair, col)
    wv_s = _q8(wv * SW)                                   # [C, C]
    wv8 = np.ascontiguousarray(
        wv_s.reshape(4, 2, P, 2, 512)                     # (i, j, p, n, e)
        .transpose(3, 2, 0, 1, 4)                         # (n, p, i, j, e)
    ).reshape(2, P, 4 * 2 * 512)

    # wproj8 [2, P, 4*2*512]
    wp_s = _q8(w_proj.astype(f32) * SW)
    wproj8 = np.ascontiguousarray(
        wp_s.reshape(4, 2, P, 2, 512).transpose(3, 2, 0, 1, 4)
    ).reshape(2, P, 4 * 2 * 512)

    # wfc1 [32, P, 4*2*2*128]: per kk: (kop i, hl, pair j, col m)
    whi, wlo = _q8_hilo(wfc_eff * SW)                     # [C, FF] each
    wfc1 = np.stack([whi, wlo], axis=0)                   # (hl, C, FF)
    wfc1 = np.ascontiguousarray(
        wfc1.reshape(2, 4, 2, P, NFF, P)                  # (hl, i, j, p, kk, m)
        .transpose(4, 3, 1, 0, 2, 5)                      # (kk, p, i, hl, j, m)
    ).reshape(NFF, P, 4 * 2 * 2 * P)

    # wfc2 [16, P, 2*2*1024]: per jj: (hl, pair i, colC) in one DMA
    w2hi, w2lo = _q8_hilo(w_fc2.astype(f32) * SW)         # [FF, C]
    wfc2 = np.stack([w2hi, w2lo], axis=0)                 # (hl, FF, C)
    wfc2 = np.ascontiguousarray(
        wfc2.reshape(2, 16, 2, P, C)                      # (hl, jj, i, p, colC)
        .transpose(1, 3, 0, 2, 4)                         # (jj, p, hl, i, colC)
    ).reshape(16, P, 2 * 2 * C)

    shared = {
        "wqk8": wqk8,
        "wv8": wv8,
        "bqk_t": bqk_t.astype(f32),
        "wproj8": wproj8,
        "bproj_b": np.broadcast_to(bproj_eff.reshape(1, C),
                                   (P, C)).astype(bf16).copy(),
        "wfc1": wfc1,
        "bfc_t": bfc_t.astype(f32),
        "wfc2": wfc2,
        "bfc2_b": np.broadcast_to(b_fc2.astype(f32).reshape(1, C),
                                  (P, C)).astype(bf16).copy(),
        "v8ones": np.full((P, NT * 16 * HD), SV, dtype=E4),
    }
    per_core = []
    t_idx = np.arange(T)
    lnSS = np.float32(np.log(SS))
    for b in range(B):
        mask = np.where(t_idx < int(seq_ls[b]), lnSS, MASK_VAL).astype(f32)
        per_core.append({
            "x": np.ascontiguousarray(x[b]).astype(f32),
            "mask_cols": mask.reshape(NT, P).T.copy(),   # [P, NT]
        })
    return shared, per_core


# --------------------------------------------------------------------------
# kernel build (single NeuronCore program, SPMD across 8 cores)
# --------------------------------------------------------------------------
def _build_nc(phases=99, repeat=1):
    nc = bacc.Bacc("TRN2", target_bir_lowering=False, debug=False,
                   num_devices=N_CORES)

    x_d = nc.dram_tensor("x", [T, C], F32, kind="ExternalInput").ap()
    mask_cols_d = nc.dram_tensor("mask_cols", [P, NT], F32,
                                 kind="ExternalInput").ap()
    wqk8_d = nc.dram_tensor("wqk8", [16, P, 8 * P], FP8,
                            kind="ExternalInput").ap()
    wv8_d = nc.dram_tensor("wv8", [2, P, 8 * 512], FP8,
                           kind="ExternalInput").ap()
    bqk_t_d = nc.dram_tensor("bqk_t", [P, 16], F32, kind="ExternalInput").ap()
    wproj8_d = nc.dram_tensor("wproj8", [2, P, 8 * 512], FP8,
                              kind="ExternalInput").ap()
    bproj_b_d = nc.dram_tensor("bproj_b", [P, C], BF16,
                               kind="ExternalInput").ap()
    wfc1_d = nc.dram_tensor("wfc1", [NFF, P, 16 * P], FP8,
                            kind="ExternalInput").ap()
    bfc_t_d = nc.dram_tensor("bfc_t", [P, NFF], F32, kind="ExternalInput").ap()
    wfc2_d = nc.dram_tensor("wfc2", [16, P, 4 * C], FP8,
                            kind="ExternalInput").ap()
    bfc2_b_d = nc.dram_tensor("bfc2_b", [P, C], BF16,
                              kind="ExternalInput").ap()
    v8ones_d = nc.dram_tensor("v8ones", [P, NT * 16 * HD], FP8,
                              kind="ExternalInput").ap()
    out_d = nc.dram_tensor("out", [T, C], F32, kind="ExternalOutput").ap()

    # DRAM access-pattern views
    x_v = x_d.rearrange("(i p) c -> p i c", p=P)          # [P, NT, C]
    out_v = out_d.rearrange("(i p) c -> p i c", p=P)
    wqk8_v = wqk8_d.rearrange("m p (i j c) -> m p i j c", i=4, j=2)
    wv8_v = wv8_d.rearrange("n p (i j c) -> n p i j c", i=4, j=2)
    wproj8_v = wproj8_d.rearrange("n p (i j c) -> n p i j c", i=4, j=2)
    wfc1_v = wfc1_d.rearrange("k p (i h j c) -> k p i h j c", i=4, h=2, j=2)
    wfc2_v = wfc2_d.rearrange("k p (h i c) -> k p h i c", h=2, i=2)

    with tile.TileContext(nc) as tc:
        with (
            tc.tile_pool(name="persist", bufs=1) as pp,
            tc.tile_pool(name="qpool", bufs=2) as qpool,
            tc.tile_pool(name="kpool", bufs=2) as kpool,
            tc.tile_pool(name="stpool", bufs=4) as stpool,
            tc.tile_pool(name="sinvb", bufs=2) as sinvbp,
            tc.tile_pool(name="small", bufs=4) as smallp,
            tc.tile_pool(name="wslab", bufs=4) as wslabp,
            tc.tile_pool(name="wrhs", bufs=2) as wrhsp,
            tc.tile_pool(name="wfc2p", bufs=6) as wfc2p,
            tc.tile_pool(name="xntok", bufs=2) as xntokp,
            tc.tile_pool(name="bigps", bufs=2, space="PSUM") as bigps,
            tc.tile_pool(name="avps", bufs=2, space="PSUM") as avps,
        ):
            try:
                for _rep in range(repeat):
                    # ---- persistent tiles ----
                    x_sb = pp.tile([P, NT, C], F32, tag="x")            # 32KB
                    xnT8 = pp.tile([P, NCD, T], FP8, tag="xnT8")        # 8KB
                    v8 = pp.tile([P, NT, 16 * P], FP8, tag="v8")        # 16KB
                    yT8 = pp.tile([P, NCD, T], FP8, tag="yT8")          # 8KB
                    h2T8 = pp.tile([P, NFF, T], FP8, tag="h2T8")        # 32KB
                    ident_f = pp.tile([P, P], F32, tag="idf")
                    bproj_b = pp.tile([P, C], BF16, tag="bprojb")
                    bfc2_b = pp.tile([P, C], BF16, tag="bfc2b")
                    bqk_t = pp.tile([P, 16], F32, tag="bqkt")
                    mask_cols = pp.tile([P, NT], F32, tag="maskc")
                    bfc_t = pp.tile([P, NFF], F32, tag="bfct")

                    make_identity(nc, ident_f)

                    # ---- load x first (LN1 pipelines behind the DMA) ----
                    for i in range(NT):
                        nc.sync.dma_start(x_sb[:, i, :], x_v[:, i, :])
                    nc.sync.dma_start(bqk_t[:], bqk_t_d)
                    nc.sync.dma_start(mask_cols[:], mask_cols_d)
                    nc.sync.dma_start(bfc_t[:], bfc_t_d)
                    nc.sync.dma_start(bproj_b[:], bproj_b_d)
                    nc.sync.dma_start(bfc2_b[:], bfc2_b_d)

                    # ---- LayerNorm: token-major stats, xn scaled xSX,
                    #      transpose, evict fp8 feature-major dstT8 ----
                    def layernorm_to_T8(dstT8, tile_cb=None):
                        for i in range(NT):
                            xi = x_sb[:, i, :]
                            stats6 = smallp.tile([P, 2, 6], F32, tag="stats6")
                            nc.vector.bn_stats(stats6[:, 0, :], xi[:, 0:512])
                            nc.vector.bn_stats(stats6[:, 1, :], xi[:, 512:1024])
                            mv = smallp.tile([P, 2], F32, tag="mv")
                            nc.vector.bn_aggr(mv[:], stats6.rearrange("p a b -> p (a b)"))
                            rstd = smallp.tile([P, 1], F32, tag="rstd")
                            nc.vector.tensor_scalar_add(rstd[:], mv[:, 1:2], EPS)
                            nc.scalar.sqrt(rstd[:], rstd[:])
                            nc.vector.reciprocal(rstd[:], rstd[:])
                            rstd16 = smallp.tile([P, 1], F32, tag="rstd16")
                            nc.vector.tensor_scalar_mul(rstd16[:], rstd[:], SX)
                            negmr = smallp.tile([P, 1], F32, tag="negmr")
                            nc.vector.scalar_tensor_tensor(
                                negmr[:], mv[:, 0:1], -1.0, rstd16[:],
                                op0=OP.mult, op1=OP.mult)
                            xn = xntokp.tile([P, C], F32, tag="xntok")
                            nc.scalar.activation(xn[:], xi, AF.Identity,
                                                 bias=negmr[:], scale=rstd16[:])
                            # transpose [P(t),C] -> feature-major dstT8[:, c, t]
                            ps = bigps.tile([P, T], F32, tag="big")
                            for j in range(NCD):
                                nc.tensor.matmul(
                                    ps[:, j * P:(j + 1) * P],
                                    xn[:, j * P:(j + 1) * P],
                                    ident_f[:], is_transpose=True,
                                    start=True, stop=True,
                                    skip_group_check=True)
                            nc.vector.tensor_copy(
                                dstT8[:, :, i * P:(i + 1) * P],
                                ps.rearrange("p (a b) -> p a b", b=P))
                            if tile_cb is not None:
                                tile_cb(i)

                    # ---- V = xn @ wv (fp8 DR, x-stationary, key-major out;
                    # per head: 64 dims + ones(=SV) cols; folded into the LN1
                    # loop per token tile since V(mt) needs only LN tile mt) --
                    v_view = v8.rearrange("p i (hh e) -> p i hh e", e=P)
                    nc.sync.dma_start(v_view[:, :, :, HD:P], v8ones_d)
                    vslabs = []
                    for n in range(2):
                        slab = wrhsp.tile([P, 4, 2, 512], FP8, tag="wrhs",
                                          name=f"wv_{n}")
                        nc.sync.dma_start(
                            slab.rearrange("p i j e -> p (i j e)"), wv8_v[n])
                        vslabs.append(slab)

                    def vgen_tile(mt):
                        if phases < 2:
                            return
                        ps = bigps.tile([P, T], F32, tag="big")
                        for n in range(2):
                            for i in range(4):
                                nc.tensor.matmul(
                                    ps[:, n * 512:(n + 1) * 512],
                                    xnT8[:, 2 * i:2 * i + 2,
                                         mt * P:(mt + 1) * P],
                                    vslabs[n][:, i], start=(i == 0),
                                    stop=(i == 3), perf_mode=DR,
                                    skip_group_check=True)
                        # ps cols: (n, pr, parity, 64): head h = 8n+2pr+q
                        psv = ps.rearrange("p (n pr two e) -> p (n pr) two e",
                                           two=2, e=HD, n=2)
                        nc.vector.tensor_scalar_mul(
                            v_view[:, mt, 0:16:2, 0:HD], psv[:, :, 0, :],
                            S_V_EV)
                        nc.vector.tensor_scalar_mul(
                            v_view[:, mt, 1:16:2, 0:HD], psv[:, :, 1, :],
                            S_V_EV)

                    layernorm_to_T8(xnT8, tile_cb=vgen_tile)

                    if phases < 3:
                        raise _PhaseDone()
                    # ---- attention ----
                    def qkgen(m):
                        q_sb = qpool.tile([P, T], BF16, tag="q", name=f"q_{m}")
                        k_sb = kpool.tile([P, T], BF16, tag="k", name=f"k_{m}")
                        for which, mm in ((0, m), (1, m + 8)):  # 0=q, 1=k
                            slab = wslabp.tile([P, 4, 2, P], FP8, tag="wslab",
                                               name=f"wqk_{m}_{which}")
                            nc.sync.dma_start(
                                slab.rearrange("p i j e -> p (i j e)"),
                                wqk8_v[mm])
                            dst = q_sb if which == 0 else k_sb
                            ps = avps.tile([P, T], F32, tag="av",
                                           name=f"gen_{m}_{which}")
                            for n in range(2):
                                for i in range(4):
                                    nc.tensor.matmul(
                                        ps[:, n * 512:(n + 1) * 512],
                                        slab[:, i],
                                        xnT8[:, 2 * i:2 * i + 2,
                                             n * 512:(n + 1) * 512],
                                        start=(i == 0), stop=(i == 3),
                                        perf_mode=DR, skip_group_check=True)
                            nc.vector.scalar_tensor_tensor(
                                dst[:], ps[:], S_QKV_EV,
                                bqk_t[:, mm:mm + 1].broadcast_to((P, T)),
                                op0=OP.mult, op1=OP.add)
                        return q_sb, k_sb

                    qk_next = qkgen(0)

                    def av_group(m_, j, st8s_, ps_ys_):
                        for hh in range(2):
                            voff = P * (2 * m_ + hh)
                            for n in range(2):
                                nc.tensor.matmul(
                                    ps_ys_[hh][:, n * 512:(n + 1) * 512],
                                    v8[:, 2 * j:2 * j + 2, voff:voff + P],
                                    st8s_[hh][:, 2 * j:2 * j + 2,
                                              n * 512:(n + 1) * 512],
                                    start=(j == 0), stop=(j == 3),
                                    perf_mode=DR, skip_group_check=True)

                    def normalize(m_, ps_ys_):
                        # sums sit in psum rows 64-127 (replicated ones cols)
                        for hh in range(2):
                            hr = slice(hh * 64, hh * 64 + 64)
                            sinv64 = sinvbp.tile([64, T], F32, tag="sinvb")
                            nc.vector.reciprocal(sinv64[:],
                                                 ps_ys_[hh][64:128, :])
                            nc.vector.scalar_tensor_tensor(
                                yT8[hr, m_, :], ps_ys_[hh][0:64, :], SY,
                                sinv64[:], op0=OP.mult, op1=OP.mult)

                    pending = None   # (m, st8s, ps_ys) with AV j=3 + norm due
                    for m in range(NH // 2):  # head pairs (2m, 2m+1)
                        q_sb, k_sb = qk_next
                        st8s = []
                        for hh in range(2):
                            st8s.append(stpool.tile([P, NT, T], FP8, tag="st",
                                                    name=f"st_{m}_{hh}"))
                        ps_ys = None
                        for kt in range(NT):
                            pss = []
                            for hh in range(2):
                                ps_kt = bigps.tile([P, T], F32, tag="big")
                                pss.append(ps_kt)
                            for n in range(2):
                                for hh in range(2):
                                    hr = slice(hh * 64, hh * 64 + 64)
                                    nc.tensor.matmul(
                                        pss[hh][:, n * 512:(n + 1) * 512],
                                        k_sb[hr, kt * P:(kt + 1) * P],
                                        q_sb[hr, n * 512:(n + 1) * 512],
                                        start=True, stop=True,
                                        skip_group_check=True)
                            if kt == 0 and pending is not None:
                                pm, pst8s, pps = pending
                                av_group(pm, 3, pst8s, pps)
                                normalize(pm, pps)
                                pending = None
                            for hh in range(2):
                                nc.scalar.activation(
                                    st8s[hh][:, kt, :], pss[hh][:], AF.Exp,
                                    bias=mask_cols[:, kt:kt + 1])
                            if kt == 0 and m + 1 < NH // 2:
                                # prefetch next pair's q/k during exp work
                                qk_next = qkgen(m + 1)
                            # AV for key-tile pair j once st8 of kt=2j+1 done
                            if kt % 2 == 1 and kt < 7:
                                if ps_ys is None:
                                    ps_ys = []
                                    for hh in range(2):
                                        ps_y = avps.tile(
                                            [P, T], F32, tag="av",
                                            name=f"av_{m}_{hh}")
                                        ps_ys.append(ps_y)
                                av_group(m, kt // 2, st8s, ps_ys)
                        pending = (m, st8s, ps_ys)
                    pm, pst8s, pps = pending
                    av_group(pm, 3, pst8s, pps)
                    normalize(pm, pps)

                    if phases < 4:
                        raise _PhaseDone()
                    # ---- residual prep: x += bproj_row (Pool engine) ----
                    for i in range(NT):
                        nc.gpsimd.tensor_tensor(x_sb[:, i, :], x_sb[:, i, :],
                                                bproj_b[:], OP.add)

                    # ---- proj: x1 = x + y @ wproj (fp8 DR, y-stationary) ----
                    pslabs = []
                    for n in range(2):
                        slab = wrhsp.tile([P, 4, 2, 512], FP8, tag="wrhs",
                                          name=f"wproj_{n}")
                        nc.sync.dma_start(
                            slab.rearrange("p i j e -> p (i j e)"), wproj8_v[n])
                        pslabs.append(slab)
                    for mt in range(NT):
                        ps = bigps.tile([P, T], F32, tag="big")
                        for n in range(2):
                            for i in range(4):
                                nc.tensor.matmul(
                                    ps[:, n * 512:(n + 1) * 512],
                                    yT8[:, 2 * i:2 * i + 2,
                                        mt * P:(mt + 1) * P],
                                    pslabs[n][:, i], start=(i == 0),
                                    stop=(i == 3), perf_mode=DR,
                                    skip_group_check=True)
                        nc.vector.scalar_tensor_tensor(
                            x_sb[:, mt, :], ps[:], S_PROJ_EV, x_sb[:, mt, :],
                            op0=OP.mult, op1=OP.add)

                    if phases < 5:
                        raise _PhaseDone()
                    # ---- LN2 -> xn2T8 (reuse xnT8 tile) ----
                    layernorm_to_T8(xnT8)

                    # ---- residual prep 2: x1 += bfc2_row (Pool engine) ----
                    for i in range(NT):
                        nc.gpsimd.tensor_tensor(x_sb[:, i, :], x_sb[:, i, :],
                                                bfc2_b[:], OP.add)

                    if phases < 6:
                        raise _PhaseDone()
                    # prefetch first FC2 rhs tiles so FC2 starts immediately
                    fc2_pre = []
                    for jj in range(2):
                        rhs = wfc2p.tile([P, 2, 2, C], FP8, tag="wfc2",
                                         name=f"fc2pre_{jj}")
                        nc.sync.dma_start(
                            rhs.rearrange("p h i e -> p (h i e)"), wfc2_v[jj])
                        fc2_pre.append(rhs)

                    # ---- FC1 (fp8 DR, weight-stationary, both token halves
                    # per stationary; hi+lo accumulate in one psum group) ----
                    for kk in range(NFF):
                        slab = wslabp.tile([P, 4, 2, 2, P], FP8, tag="wfc1")
                        nc.sync.dma_start(
                            slab.rearrange("p i h j e -> p (i h j e)"),
                            wfc1_v[kk])
                        ps = bigps.tile([P, T], F32, tag="big")
                        for i in range(4):
                            for hl in range(2):
                                for th in range(2):
                                    nc.tensor.matmul(
                                        ps[:, th * 512:(th + 1) * 512],
                                        slab[:, i, hl],
                                        xnT8[:, 2 * i:2 * i + 2,
                                             th * 512:(th + 1) * 512],
                                        start=(i == 0 and hl == 0),
                                        stop=(i == 3 and hl == 1),
                                        perf_mode=DR, skip_group_check=True)
                        nc.scalar.activation(
                            h2T8[:, kk, :], ps[:], AF.Relu,
                            bias=bfc_t[:, kk:kk + 1], scale=S_FC1_EV)

                    # ---- FC2 (fp8 DR, h2-stationary, both column halves per
                    # stationary; hi+lo in one psum group), per token half ----
                    for th in range(2):
                        big4 = [bigps.tile([P, T], F32, tag="big",
                                           name=f"fc2ps_{th}_{jj}")
                                for jj in range(2)] + \
                               [avps.tile([P, T], F32, tag="av",
                                          name=f"fc2av_{th}_{jj}")
                                for jj in range(2)]
                        slots = [big4[0][:, 0:512], big4[0][:, 512:1024],
                                 big4[1][:, 0:512], big4[1][:, 512:1024],
                                 big4[2][:, 0:512], big4[2][:, 512:1024],
                                 big4[3][:, 0:512], big4[3][:, 512:1024]]
                        # accumulator (ch, mt) -> slots[4*ch + mt]
                        pss = [[slots[4 * ch + mt] for mt in range(4)]
                               for ch in range(2)]
                        for jj in range(16):
                            if th == 0 and jj < 2:
                                rhs = fc2_pre[jj]
                            else:
                                rhs = wfc2p.tile([P, 2, 2, C], FP8,
                                                 tag="wfc2")
                                nc.sync.dma_start(
                                    rhs.rearrange("p h i e -> p (h i e)"),
                                    wfc2_v[jj])
                            for hl in range(2):
                                for mt in range(4):
                                    tok = (4 * th + mt) * P
                                    for ch in range(2):
                                        nc.tensor.matmul(
                                            pss[ch][mt],
                                            h2T8[:, 2 * jj:2 * jj + 2,
                                                 tok:tok + P],
                                            rhs[:, hl, :,
                                                ch * 512:(ch + 1) * 512],
                                            start=(jj == 0 and hl == 0),
                                            stop=(jj == 15 and hl == 1),
                                            perf_mode=DR,
                                            skip_group_check=True)
                                    if jj == 15 and hl == 1:
                                        # evict + write out this token tile now
                                        i = 4 * th + mt
                                        for ch in range(2):
                                            nc.vector.scalar_tensor_tensor(
                                                x_sb[:, i,
                                                     ch * 512:(ch + 1) * 512],
                                                pss[ch][mt], S_FC2_EV,
                                                x_sb[:, i,
                                                     ch * 512:(ch + 1) * 512],
                                                op0=OP.mult, op1=OP.add)
                                        nc.sync.dma_start(out_v[:, i, :],
                                                          x_sb[:, i, :])
            except _PhaseDone:
                pass

    nc.compile()
    return nc


class _PhaseDone(Exception):
    pass


_NC_CACHE = None


def _get_nc():
    global _NC_CACHE
    if _NC_CACHE is None:
        _NC_CACHE = _build_nc()
    return _NC_CACHE


def _run(inputs, trace=False, **kwargs):
    shared, per_core = _host_prep(**inputs)
    nc = _get_nc()
    in_maps = [{**shared, **pc} for pc in per_core]
    res = run_bass_kernel_spmd(nc, in_maps, core_ids=list(range(N_CORES)),
                               trace=trace, **kwargs)
    out = np.stack([res.results[i]["out"] for i in range(N_CORES)], axis=0)
    return out.astype(np.float32), res


def kernel(**inputs):
    return _run(inputs)[0]


# revision 25
# speedup vs baseline: 2.1522x; 1.0563x over previous
"""Trainium2 Bass kernel for a dense transformer block (pre-LN, masked attention).

Sharding: data-parallel over batch B=8 across the 8 NeuronCores — each core
processes one full batch element [T=1024, C=1024]; weights are replicated.
No collectives needed.

Per-core dataflow (single NeuronCore), fp8-heavy:
  - LN1 token-major stats; normalized xn scaled x16 (bf16), PE-transposed,
    evicted to fp8 xnT8 [C, T] (e4m3, DVE copy).
  - QKV in fp8 DoubleRow (contraction pairs of 128-blocks): weights
    pre-quantized x512 on host.  Q/K evicted bf16 in true units (DVE, bias
    broadcast along free dim).  V evicted fp8 x32 key-major with a ones(=32)
    column per head so AV's psum row 64 gives the softmax sums.
  - QK^T bf16 (contraction = 64 head dims): the two heads of a pair occupy
    partitions 0-63 / 64-127, issued interleaved so their matmuls run
    concurrently in different row-groups of the PE array (tile_position
    auto-derived).  exp on ScalarE ([128,1024] ops) with key-padding mask +
    ln(8) bias, evicted straight to fp8 st8 (x8).
  - AV in fp8 DoubleRow over key-tile pairs; normalization fused into the
    yT8 eviction (x512, DVE scalar_tensor_tensor with broadcast 1/rowsum).
  - proj in fp8 DoubleRow, eviction fused with residual add (DVE).
  - LN2 -> xn2T8 (same path as LN1).
  - FC1/FC2 in fp8 DoubleRow with same-scale hi/lo weight split (w = fp8(w)
    + fp8(w - fp8(w)): residual lands in e4m3 denormals, so weights carry
    ~bf16 accuracy; both halves accumulate in one PSUM group).  FC1 is
    weight-stationary (each stationary reused for both token halves); FC2
    token-major (each h2T8 stationary reused for both output-column halves).
  - PSUM: one pool of 3 x [P,1024] (2-bank) tiles + one of 2 x [P,512];
    evictions run on [P,1024] at once to halve per-op overheads.
"""

import os
import sys
import numpy as np
import ml_dtypes

for _p in ("/opt/trn_rl_repo", "/opt/pypackages"):
    if os.path.isdir(_p) and _p not in sys.path:
        sys.path.append(_p)

import concourse.bass as bass
import concourse.mybir as mybir
import concourse.tile as tile
from concourse import bacc
from concourse.bass_utils import run_bass_kernel_spmd
from concourse.masks import make_identity

P = 128
B, T, C = 8, 1024, 1024
NH, HD = 16, 64
FF = 4 * C
EPS = 1e-5
NT = T // P      # 8 token tiles
NCD = C // P     # 8 feature tiles
NFF = FF // P    # 32 ff tiles
N_CORES = 8
MASK_VAL = -30000.0

F32 = mybir.dt.float32
BF16 = mybir.dt.bfloat16
FP8 = mybir.dt.float8e4
AF = mybir.ActivationFunctionType
OP = mybir.AluOpType
DR = mybir.MatmulPerfMode.DoubleRow

bf16 = ml_dtypes.bfloat16
E4 = ml_dtypes.float8_e4m3

# power-of-2 quantization scales
SX = 16.0     # normalized activations (xn, xn2)
SW = 512.0    # weights
SV = 32.0     # v
SS = 8.0      # st = exp(logits)
SY = 512.0    # y (attention out)
SH = 32.0     # h2 scale
S_QKV_EV = 1.0 / (SX * SW)          # q/k eviction: psum -> true units
S_V_EV = SV / (SX * SW)             # v eviction
S_PROJ_EV = 1.0 / (SY * SW)         # proj eviction
S_FC1_EV = SH / (SX * SW)           # fc1 eviction: h2*32 = relu(ps/256+32b)
S_FC2_EV = 1.0 / (SH * SW)          # fc2 eviction


def _q8(a):
    return np.clip(a, -240, 240).astype(E4)


def _q8_hilo(a):
    """same-scale hi/lo split: returns (hi, lo) fp8 arrays; hi+lo ~ a."""
    hi = _q8(a)
    lo = _q8(a - hi.astype(np.float32))
    return hi, lo


# --------------------------------------------------------------------------
# host-side preparation: fold LN gains/biases into weights, quantize to fp8
# --------------------------------------------------------------------------
def _host_prep(x, seq_ls, ln1_g, ln1_b, w_qkv, b_qkv, w_proj, b_proj,
               ln2_g, ln2_b, w_fc, b_fc, w_fc2, b_fc2):
    f32 = np.float32
    ln1_g, ln1_b = ln1_g.astype(f32), ln1_b.astype(f32)
    w_qkv = w_qkv.astype(f32)

    wqkv_eff = ln1_g[:, None] * w_qkv                     # [C, 3C]
    bqkv_eff = ln1_b @ w_qkv + b_qkv.astype(f32)          # [3C]
    scale = np.float32(1.0 / np.sqrt(HD))
    wq = wqkv_eff[:, :C] * scale
    bq = bqkv_eff[:C] * scale
    wk = wqkv_eff[:, C:2 * C]
    bk = bqkv_eff[C:2 * C]
    wv = wqkv_eff[:, 2 * C:]
    bv = bqkv_eff[2 * C:]

    bproj_eff = bv @ w_proj.astype(f32) + b_proj.astype(f32)   # [C]

    wfc_eff = ln2_g.astype(f32)[:, None] * w_fc.astype(f32)    # [C, FF]
    bfc_eff = ln2_b.astype(f32) @ w_fc.astype(f32) + b_fc.astype(f32)

    wqk = np.concatenate([wq, wk], axis=1)                # [C, 2C]
    bqk_t = np.concatenate([bq, bk]).reshape(16, P).T.copy()   # [P, 16]
    bfc_t = (bfc_eff * SH).reshape(NFF, P).T.copy()       # [P, 32] (x32)

    # --- fp8 weight layouts ---
    # wqk8 [16, P, 4*2*128]: per head-pair column block mm: (kop, pair, col)
    wqk_s = _q8(wqk * SW)                                 # [C, 2C]
    wqk8 = np.ascontiguousarray(
        wqk_s.reshape(4, 2, P, 16, P)                     # (i, j, p, mm, m)
        .transpose(3, 2, 0, 1, 4)                         # (mm, p, i, j, m)
    ).reshape(16, P, 4 * 2 * P)

    # wv8 [2, P, 4*2*512]: slab n covers output cols n*512..: (kop, pair, col)
    wv_s = _q8(wv * SW)                                   # [C, C]
    wv8 = np.ascontiguousarray(
        wv_s.reshape(4, 2, P, 2, 512)                     # (i, j, p, n, e)
        .transpose(3, 2, 0, 1, 4)                         # (n, p, i, j, e)
    ).reshape(2, P, 4 * 2 * 512)

    # wproj8 [2, P, 4*2*512]
    wp_s = _q8(w_proj.astype(f32) * SW)
    wproj8 = np.ascontiguousarray(
        wp_s.reshape(4, 2, P, 2, 512).transpose(3, 2, 0, 1, 4)
    ).reshape(2, P, 4 * 2 * 512)

    # wfc1 [32, P, 4*2*2*128]: per kk: (kop i, hl, pair j, col m)
    whi, wlo = _q8_hilo(wfc_eff * SW)                     # [C, FF] each
    wfc1 = np.stack([whi, wlo], axis=0)                   # (hl, C, FF)
    wfc1 = np.ascontiguousarray(
        wfc1.reshape(2, 4, 2, P, NFF, P)                  # (hl, i, j, p, kk, m)
        .transpose(4, 3, 1, 0, 2, 5)                      # (kk, p, i, hl, j, m)
    ).reshape(NFF, P, 4 * 2 * 2 * P)

    # wfc2 [16, P, 2*2*1024]: per jj: (hl, pair i, colC) in one DMA
    w2hi, w2lo = _q8_hilo(w_fc2.astype(f32) * SW)         # [FF, C]
    wfc2 = np.stack([w2hi, w2lo], axis=0)                 # (hl, FF, C)
    wfc2 = np.ascontiguousarray(
        wfc2.reshape(2, 16, 2, P, C)                      # (hl, jj, i, p, colC)
        .transpose(1, 3, 0, 2, 4)                         # (jj, p, hl, i, colC)
    ).reshape(16, P, 2 * 2 * C)

    shared = {
        "wqk8": wqk8,
        "wv8": wv8,
        "bqk_t": bqk_t.astype(f32),
        "wproj8": wproj8,
        "bproj_b": np.broadcast_to(bproj_eff.reshape(1, C),
                                   (P, C)).astype(bf16).copy(),
        "wfc1": wfc1,
        "bfc_t": bfc_t.astype(f32),
        "wfc2": wfc2,
        "bfc2_b": np.broadcast_to(b_fc2.astype(f32).reshape(1, C),
                                  (P, C)).astype(bf16).copy(),
        "v8ones": np.full((P, NT * 16 * HD), SV, dtype=E4),
    }
    per_core = []
    t_idx = np.arange(T)
    lnSS = np.float32(np.log(SS))
    for b in range(B):
        mask = np.where(t_idx < int(seq_ls[b]), lnSS, MASK_VAL).astype(f32)
        per_core.append({
            "x": np.ascontiguousarray(x[b]).astype(f32),
            "mask_cols": mask.reshape(NT, P).T.copy(),   # [P, NT]
        })
    return shared, per_core


# --------------------------------------------------------------------------
# kernel build (single NeuronCore program, SPMD across 8 cores)
# --------------------------------------------------------------------------
def _build_nc(phases=99, repeat=1):
    nc = bacc.Bacc("TRN2", target_bir_lowering=False, debug=False,
                   num_devices=N_CORES)

    x_d = nc.dram_tensor("x", [T, C], F32, kind="ExternalInput").ap()
    mask_cols_d = nc.dram_tensor("mask_cols", [P, NT], F32,
                                 kind="ExternalInput").ap()
    wqk8_d = nc.dram_tensor("wqk8", [16, P, 8 * P], FP8,
                            kind="ExternalInput").ap()
    wv8_d = nc.dram_tensor("wv8", [2, P, 8 * 512], FP8,
                           kind="ExternalInput").ap()
    bqk_t_d = nc.dram_tensor("bqk_t", [P, 16], F32, kind="ExternalInput").ap()
    wproj8_d = nc.dram_tensor("wproj8", [2, P, 8 * 512], FP8,
                              kind="ExternalInput").ap()
    bproj_b_d = nc.dram_tensor("bproj_b", [P, C], BF16,
                               kind="ExternalInput").ap()
    wfc1_d = nc.dram_tensor("wfc1", [NFF, P, 16 * P], FP8,
                            kind="ExternalInput").ap()
    bfc_t_d = nc.dram_tensor("bfc_t", [P, NFF], F32, kind="ExternalInput").ap()
    wfc2_d = nc.dram_tensor("wfc2", [16, P, 4 * C], FP8,
                            kind="ExternalInput").ap()
    bfc2_b_d = nc.dram_tensor("bfc2_b", [P, C], BF16,
                              kind="ExternalInput").ap()
    v8ones_d = nc.dram_tensor("v8ones", [P, NT * 16 * HD], FP8,
                              kind="ExternalInput").ap()
    out_d = nc.dram_tensor("out", [T, C], F32, kind="ExternalOutput").ap()

    # DRAM access-pattern views
    x_v = x_d.rearrange("(i p) c -> p i c", p=P)          # [P, NT, C]
    out_v = out_d.rearrange("(i p) c -> p i c", p=P)
    wqk8_v = wqk8_d.rearrange("m p (i j c) -> m p i j c", i=4, j=2)
    wv8_v = wv8_d.rearrange("n p (i j c) -> n p i j c", i=4, j=2)
    wproj8_v = wproj8_d.rearrange("n p (i j c) -> n p i j c", i=4, j=2)
    wfc1_v = wfc1_d.rearrange("k p (i h j c) -> k p i h j c", i=4, h=2, j=2)
    wfc2_v = wfc2_d.rearrange("k p (h i c) -> k p h i c", h=2, i=2)

    with tile.TileContext(nc) as tc:
        with (
            tc.tile_pool(name="persist", bufs=1) as pp,
            tc.tile_pool(name="qpool", bufs=2) as qpool,
            tc.tile_pool(name="kpool", bufs=2) as kpool,
            tc.tile_pool(name="stpool", bufs=4) as stpool,
            tc.tile_pool(name="sinvb", bufs=2) as sinvbp,
            tc.tile_pool(name="small", bufs=4) as smallp,
            tc.tile_pool(name="wslab", bufs=4) as wslabp,
            tc.tile_pool(name="wrhs", bufs=2) as wrhsp,
            tc.tile_pool(name="wfc2p", bufs=6) as wfc2p,
            tc.tile_pool(name="xntok", bufs=2) as xntokp,
            tc.tile_pool(name="bigps", bufs=2, space="PSUM") as bigps,
            tc.tile_pool(name="avps", bufs=2, space="PSUM") as avps,
        ):
            try:
                for _rep in range(repeat):
                    # ---- persistent tiles ----
                    x_sb = pp.tile([P, NT, C], F32, tag="x")            # 32KB
                    xnT8 = pp.tile([P, NCD, T], FP8, tag="xnT8")        # 8KB
                    v8 = pp.tile([P, NT, 16 * P], FP8, tag="v8")        # 16KB
                    yT8 = pp.tile([P, NCD, T], FP8, tag="yT8")          # 8KB
                    h2T8 = pp.tile([P, NFF, T], FP8, tag="h2T8")        # 32KB
                    ident_f = pp.tile([P, P], F32, tag="idf")
                    bproj_b = pp.tile([P, C], BF16, tag="bprojb")
                    bfc2_b = pp.tile([P, C], BF16, tag="bfc2b")
                    bqk_t = pp.tile([P, 16], F32, tag="bqkt")
                    mask_cols = pp.tile([P, NT], F32, tag="maskc")
                    bfc_t = pp.tile([P, NFF], F32, tag="bfct")

                    make_identity(nc, ident_f)

                    # ---- load x first (LN1 pipelines behind the DMA) ----
                    for i in range(NT):
                        nc.sync.dma_start(x_sb[:, i, :], x_v[:, i, :])
                    nc.sync.dma_start(bqk_t[:], bqk_t_d)
                    nc.sync.dma_start(mask_cols[:], mask_cols_d)
                    nc.sync.dma_start(bfc_t[:], bfc_t_d)
                    nc.sync.dma_start(bproj_b[:], bproj_b_d)
                    nc.sync.dma_start(bfc2_b[:], bfc2_b_d)

                    # ---- LayerNorm: token-major stats, xn scaled xSX,
                    #      transpose, evict fp8 feature-major dstT8 ----
                    def layernorm_to_T8(dstT8):
                        for i in range(NT):
                            xi = x_sb[:, i, :]
                            stats6 = smallp.tile([P, 2, 6], F32, tag="stats6")
                            nc.vector.bn_stats(stats6[:, 0, :], xi[:, 0:512])
                            nc.vector.bn_stats(stats6[:, 1, :], xi[:, 512:1024])
                            mv = smallp.tile([P, 2], F32, tag="mv")
                            nc.vector.bn_aggr(mv[:], stats6.rearrange("p a b -> p (a b)"))
                            rstd = smallp.tile([P, 1], F32, tag="rstd")
                            nc.vector.tensor_scalar_add(rstd[:], mv[:, 1:2], EPS)
                            nc.scalar.sqrt(rstd[:], rstd[:])
                            nc.vector.reciprocal(rstd[:], rstd[:])
                            rstd16 = smallp.tile([P, 1], F32, tag="rstd16")
                            nc.vector.tensor_scalar_mul(rstd16[:], rstd[:], SX)
                            negmr = smallp.tile([P, 1], F32, tag="negmr")
                            nc.vector.scalar_tensor_tensor(
                                negmr[:], mv[:, 0:1], -1.0, rstd16[:],
                                op0=OP.mult, op1=OP.mult)
                            xn = xntokp.tile([P, C], F32, tag="xntok")
                            nc.scalar.activation(xn[:], xi, AF.Identity,
                                                 bias=negmr[:], scale=rstd16[:])
                            # transpose [P(t),C] -> feature-major dstT8[:, c, t]
                            ps = bigps.tile([P, T], F32, tag="big")
                            for j in range(NCD):
                                nc.tensor.matmul(
                                    ps[:, j * P:(j + 1) * P],
                                    xn[:, j * P:(j + 1) * P],
                                    ident_f[:], is_transpose=True,
                                    start=True, stop=True,
                                    skip_group_check=True)
                            nc.vector.tensor_copy(
                                dstT8[:, :, i * P:(i + 1) * P],
                                ps.rearrange("p (a b) -> p a b", b=P))

                    layernorm_to_T8(xnT8)

                    # ---- V = xn @ wv (fp8 DR, x-stationary, key-major out;
                    # per head: 64 dims + ones(=SV) col at slot h*80+64) ----
                    if phases < 2:
                        raise _PhaseDone()
                    v_view = v8.rearrange("p i (hh e) -> p i hh e", e=P)
                    nc.sync.dma_start(v_view[:, :, :, HD:P], v8ones_d)
                    vslabs = []
                    for n in range(2):
                        slab = wrhsp.tile([P, 4, 2, 512], FP8, tag="wrhs",
                                          name=f"wv_{n}")
                        nc.sync.dma_start(
                            slab.rearrange("p i j e -> p (i j e)"), wv8_v[n])
                        vslabs.append(slab)
                    for mt in range(NT):
                        ps = bigps.tile([P, T], F32, tag="big")
                        for n in range(2):
                            for i in range(4):
                                nc.tensor.matmul(
                                    ps[:, n * 512:(n + 1) * 512],
                                    xnT8[:, 2 * i:2 * i + 2,
                                         mt * P:(mt + 1) * P],
                                    vslabs[n][:, i], start=(i == 0),
                                    stop=(i == 3), perf_mode=DR,
                                    skip_group_check=True)
                        # ps cols: (n, pr, parity, 64): head h = 8n+2pr+q
                        psv = ps.rearrange("p (n pr two e) -> p (n pr) two e",
                                           two=2, e=HD, n=2)
                        nc.vector.tensor_scalar_mul(
                            v_view[:, mt, 0:16:2, 0:HD], psv[:, :, 0, :],
                            S_V_EV)
                        nc.vector.tensor_scalar_mul(
                            v_view[:, mt, 1:16:2, 0:HD], psv[:, :, 1, :],
                            S_V_EV)

                    if phases < 3:
                        raise _PhaseDone()
                    # ---- attention ----
                    def qkgen(m):
                        q_sb = qpool.tile([P, T], BF16, tag="q", name=f"q_{m}")
                        k_sb = kpool.tile([P, T], BF16, tag="k", name=f"k_{m}")
                        for which, mm in ((0, m), (1, m + 8)):  # 0=q, 1=k
                            slab = wslabp.tile([P, 4, 2, P], FP8, tag="wslab",
                                               name=f"wqk_{m}_{which}")
                            nc.sync.dma_start(
                                slab.rearrange("p i j e -> p (i j e)"),
                                wqk8_v[mm])
                            dst = q_sb if which == 0 else k_sb
                            ps = avps.tile([P, T], F32, tag="av",
                                           name=f"gen_{m}_{which}")
                            for n in range(2):
                                for i in range(4):
                                    nc.tensor.matmul(
                                        ps[:, n * 512:(n + 1) * 512],
                                        slab[:, i],
                                        xnT8[:, 2 * i:2 * i + 2,
                                             n * 512:(n + 1) * 512],
                                        start=(i == 0), stop=(i == 3),
                                        perf_mode=DR, skip_group_check=True)
                            nc.vector.scalar_tensor_tensor(
                                dst[:], ps[:], S_QKV_EV,
                                bqk_t[:, mm:mm + 1].broadcast_to((P, T)),
                                op0=OP.mult, op1=OP.add)
                        return q_sb, k_sb

                    qk_next = qkgen(0)

                    def av_group(m_, j, st8s_, ps_ys_):
                        for hh in range(2):
                            voff = P * (2 * m_ + hh)
                            for n in range(2):
                                nc.tensor.matmul(
                                    ps_ys_[hh][:, n * 512:(n + 1) * 512],
                                    v8[:, 2 * j:2 * j + 2, voff:voff + P],
                                    st8s_[hh][:, 2 * j:2 * j + 2,
                                              n * 512:(n + 1) * 512],
                                    start=(j == 0), stop=(j == 3),
                                    perf_mode=DR, skip_group_check=True)

                    def normalize(m_, ps_ys_):
                        # sums sit in psum rows 64-127 (replicated ones cols)
                        for hh in range(2):
                            hr = slice(hh * 64, hh * 64 + 64)
                            sinv64 = sinvbp.tile([64, T], F32, tag="sinvb")
                            nc.vector.reciprocal(sinv64[:],
                                                 ps_ys_[hh][64:128, :])
                            nc.vector.scalar_tensor_tensor(
                                yT8[hr, m_, :], ps_ys_[hh][0:64, :], SY,
                                sinv64[:], op0=OP.mult, op1=OP.mult)

                    pending = None   # (m, st8s, ps_ys) with AV j=3 + norm due
                    for m in range(NH // 2):  # head pairs (2m, 2m+1)
                        q_sb, k_sb = qk_next
                        st8s = []
                        for hh in range(2):
                            st8s.append(stpool.tile([P, NT, T], FP8, tag="st",
                                                    name=f"st_{m}_{hh}"))
                        ps_ys = None
                        for kt in range(NT):
                            pss = []
                            for hh in range(2):
                                ps_kt = bigps.tile([P, T], F32, tag="big")
                                pss.append(ps_kt)
                            for n in range(2):
                                for hh in range(2):
                                    hr = slice(hh * 64, hh * 64 + 64)
                                    nc.tensor.matmul(
                                        pss[hh][:, n * 512:(n + 1) * 512],
                                        k_sb[hr, kt * P:(kt + 1) * P],
                                        q_sb[hr, n * 512:(n + 1) * 512],
                                        start=True, stop=True,
                                        skip_group_check=True)
                            if kt == 0 and pending is not None:
                                pm, pst8s, pps = pending
                                av_group(pm, 3, pst8s, pps)
                                normalize(pm, pps)
                                pending = None
                            for hh in range(2):
                                nc.scalar.activation(
                                    st8s[hh][:, kt, :], pss[hh][:], AF.Exp,
                                    bias=mask_cols[:, kt:kt + 1])
                            if kt == 0 and m + 1 < NH // 2:
                                # prefetch next pair's q/k during exp work
                                qk_next = qkgen(m + 1)
                            # AV for key-tile pair j once st8 of kt=2j+1 done
                            if kt % 2 == 1 and kt < 7:
                                if ps_ys is None:
                                    ps_ys = []
                                    for hh in range(2):
                                        ps_y = avps.tile(
                                            [P, T], F32, tag="av",
                                            name=f"av_{m}_{hh}")
                                        ps_ys.append(ps_y)
                                av_group(m, kt // 2, st8s, ps_ys)
                        pending = (m, st8s, ps_ys)
                    pm, pst8s, pps = pending
                    av_group(pm, 3, pst8s, pps)
                    normalize(pm, pps)

                    if phases < 4:
                        raise _PhaseDone()
                    # ---- residual prep: x += bproj_row (Pool engine) ----
                    for i in range(NT):
                        nc.gpsimd.tensor_tensor(x_sb[:, i, :], x_sb[:, i, :],
                                                bproj_b[:], OP.add)

                    # ---- proj: x1 = x + y @ wproj (fp8 DR, y-stationary) ----
                    pslabs = []
                    for n in range(2):
                        slab = wrhsp.tile([P, 4, 2, 512], FP8, tag="wrhs",
                                          name=f"wproj_{n}")
                        nc.sync.dma_start(
                            slab.rearrange("p i j e -> p (i j e)"), wproj8_v[n])
                        pslabs.append(slab)
                    for mt in range(NT):
                        ps = bigps.tile([P, T], F32, tag="big")
                        for n in range(2):
                            for i in range(4):
                                nc.tensor.matmul(
                                    ps[:, n * 512:(n + 1) * 512],
                                    yT8[:, 2 * i:2 * i + 2,
                                        mt * P:(mt + 1) * P],
                                    pslabs[n][:, i], start=(i == 0),
                                    stop=(i == 3), perf_mode=DR,
                                    skip_group_check=True)
                        nc.vector.scalar_tensor_tensor(
                            x_sb[:, mt, :], ps[:], S_PROJ_EV, x_sb[:, mt, :],
                            op0=OP.mult, op1=OP.add)

                    if phases < 5:
                        raise _PhaseDone()
                    # ---- LN2 -> xn2T8 (reuse xnT8 tile) ----
                    layernorm_to_T8(xnT8)

                    # ---- residual prep 2: x1 += bfc2_row (Pool engine) ----
                    for i in range(NT):
                        nc.gpsimd.tensor_tensor(x_sb[:, i, :], x_sb[:, i, :],
                                                bfc2_b[:], OP.add)

                    if phases < 6:
                        raise _PhaseDone()
                    # prefetch first FC2 rhs tiles so FC2 starts immediately
                    fc2_pre = []
                    for jj in range(2):
                        rhs = wfc2p.tile([P, 2, 2, C], FP8, tag="wfc2",
                                         name=f"fc2pre_{jj}")
                        nc.sync.dma_start(
                            rhs.rearrange("p h i e -> p (h i e)"), wfc2_v[jj])
                        fc2_pre.append(rhs)

                    # ---- FC1 (fp8 DR, weight-stationary, both token halves
                    # per stationary; hi+lo accumulate in one psum group) ----
                    for kk in range(NFF):
                        slab = wslabp.tile([P, 4, 2, 2, P], FP8, tag="wfc1")
                        nc.sync.dma_start(
                            slab.rearrange("p i h j e -> p (i h j e)"),
                            wfc1_v[kk])
                        ps = bigps.tile([P, T], F32, tag="big")
                        for i in range(4):
                            for hl in range(2):
                                for th in range(2):
                                    nc.tensor.matmul(
                                        ps[:, th * 512:(th + 1) * 512],
                                        slab[:, i, hl],
                                        xnT8[:, 2 * i:2 * i + 2,
                                             th * 512:(th + 1) * 512],
                                        start=(i == 0 and hl == 0),
                                        stop=(i == 3 and hl == 1),
                                        perf_mode=DR, skip_group_check=True)
                        nc.scalar.activation(
                            h2T8[:, kk, :], ps[:], AF.Relu,
                            bias=bfc_t[:, kk:kk + 1], scale=S_FC1_EV)

                    # ---- FC2 (fp8 DR, h2-stationary, both column halves per
                    # stationary; hi+lo in one psum group), per token half ----
                    for th in range(2):
                        big4 = [bigps.tile([P, T], F32, tag="big",
                                           name=f"fc2ps_{th}_{jj}")
                                for jj in range(2)] + \
                               [avps.tile([P, T], F32, tag="av",
                                          name=f"fc2av_{th}_{jj}")
                                for jj in range(2)]
                        slots = [big4[0][:, 0:512], big4[0][:, 512:1024],
                                 big4[1][:, 0:512], big4[1][:, 512:1024],
                                 big4[2][:, 0:512], big4[2][:, 512:1024],
                                 big4[3][:, 0:512], big4[3][:, 512:1024]]
                        # accumulator (ch, mt) -> slots[4*ch + mt]
                        pss = [[slots[4 * ch + mt] for mt in range(4)]
                               for ch in range(2)]
                        for jj in range(16):
                            if th == 0 and jj < 2:
                                rhs = fc2_pre[jj]
                            else:
                                rhs = wfc2p.tile([P, 2, 2, C], FP8,
                                                 tag="wfc2")
                                nc.sync.dma_start(
                                    rhs.rearrange("p h i e -> p (h i e)"),
                                    wfc2_v[jj])
                            for hl in range(2):
                                for mt in range(4):
                                    tok = (4 * th + mt) * P
                                    for ch in range(2):
                                        nc.tensor.matmul(
                                            pss[ch][mt],
                                            h2T8[:, 2 * jj:2 * jj + 2,
                                                 tok:tok + P],
                                            rhs[:, hl, :,
                                                ch * 512:(ch + 1) * 512],
                                            start=(jj == 0 and hl == 0),
                                            stop=(jj == 15 and hl == 1),
                                            perf_mode=DR,
                                            skip_group_check=True)
                                    if jj == 15 and hl == 1:
                                        # evict + write out this token tile now
                                        i = 4 * th + mt
                                        for ch in range(2):
                                            nc.vector.scalar_tensor_tensor(
                                                x_sb[:, i,
                                                     ch * 512:(ch + 1) * 512],
                                                pss[ch][mt], S_FC2_EV,
                                                x_sb[:, i,
                                                     ch * 512:(ch + 1) * 512],
                                                op0=OP.mult, op1=OP.add)
                                        nc.sync.dma_start(out_v[:, i, :],
                                                          x_sb[:, i, :])
            except _PhaseDone:
                pass

    nc.compile()
    return nc


class _PhaseDone(Exception):
    pass


_NC_CACHE = None


def _get_nc():
    global _NC_CACHE
    if _NC_CACHE is None:
        _NC_CACHE = _build_nc()
    return _NC_CACHE


def _run(inputs, trace=False, **kwargs):
    shared, per_core = _host_prep(**inputs)
    nc = _get_nc()
    in_maps = [{**shared, **pc} for pc in per_core]
    res = run_bass_kernel_spmd(nc, in_maps, core_ids=list(range(N_CORES)),
                               trace=trace, **kwargs)
    out = np.stack([res.results[i]["out"] for i in range(N_CORES)], axis=0)
    return out.astype(np.float32), res


def kernel(**inputs):
    return _run(inputs)[0]
